# revision 3
# baseline (speedup 1.0000x reference)
"""Trainium2 Bass kernel for nn_MultiHeadAttention_48086453846410.

Reference computation (heads folded into the sequence axis, softmax over the
FULL L = seq*heads key axis):
    qp = (q @ wk_w.T + wk_b).reshape(bs, L, d)   # note swapped wk/wq, faithful
    kp = (k @ wq_w.T + wq_b).reshape(bs, L, d)
    vp = (v @ wv_w.T + wv_b).reshape(bs, L, d)
    scores = qp @ kp.T / sqrt(d); attn = softmax(scores, -1)
    o = (attn @ vp).reshape(bs, seq, d*heads)
    out = o @ out_w.T + out_b

Sharding: 8 cores = (batch b in 0..3) x (seq half). Each core owns 256 query
seq positions of one batch (2048 query rows l' = h*256+s). Softmax is over
keys, so query rows are independent -> no collectives.

On-device layout strategy (all matmuls bf16 inputs, fp32 PSUM accumulate):
 - host pre-transposes activations/weights so no on-device transposes at all
 - qpT/kpT computed in transposed layout (proj dim j on partitions)
 - vp computed in natural layout (t on partitions)
 - scores computed transposed: scoresT[m=(g,t), l'] -> softmax needs only
   exp (scores bounded, max-subtraction provably unnecessary: |s| < 2) and
   the denominator Z, computed by a ones-matmul over partitions (replicated
   across 128 partitions for free); normalization deferred to oT columns.
 - attn@v consumes exp tiles directly as the moving operand -> oT (e on
   partitions), which is exactly the lhsT layout for the out projection.
"""

import math
import sys

for _p in ("/opt/trn_rl_repo",):
    if _p not in sys.path:
        sys.path.insert(0, _p)

import numpy as np
import ml_dtypes

BS, SEQ, D, HEADS = 4, 512, 512, 8
NCORES = 8
S = SEQ // 2            # 256 query seq rows per core
JT = HEADS * D // 128   # 32 tiles of the 4096 projection dim
DT = D // 128           # 4 tiles of the 512 contraction dim
TT = SEQ // 128         # 4 key-seq tiles per head
LSLICES = 4             # l' = 2048 per core, processed in 4 slices of 512
NP_BF16 = ml_dtypes.bfloat16

_CACHE = {}


def _build_program():
    from concourse import bacc
    import concourse.mybir as mybir
    import concourse.tile as tile
    from concourse.dt import dt

    f32 = dt.float32
    b16 = dt.bfloat16
    Act = mybir.ActivationFunctionType

    nc = bacc.Bacc(None, target_bir_lowering=False, debug=False,
                   num_devices=NCORES)

    def din(name, shape, dty=b16):
        return nc.dram_tensor(name, shape, dty, kind="ExternalInput").ap()

    qT = din("qT", [D, S])                 # q[b, half].T      (d, s)
    kT = din("kT", [D, SEQ])               # k[b].T            (d, t)
    vT = din("vT", [D, SEQ])               # v[b].T            (d, t)
    wkT = din("wkT", [D, HEADS * D])       # wk_w.T            (d, j)
    wqT = din("wqT", [D, HEADS * D])       # wq_w.T            (d, j)
    wvT = din("wvT", [D, HEADS * D])       # wv_w.T            (d, j)
    owT = din("owT", [HEADS * D, D])       # out_w.T           (c, r)
    wk_bT = din("wk_bT", [128, JT], f32)   # wk_b.reshape(JT,128).T
    wq_bT = din("wq_bT", [128, JT], f32)
    wv_br = din("wv_br", [128, HEADS * D], f32)   # wv_b replicated
    out_br = din("out_br", [128, D], f32)         # out_b replicated
    ones = din("ones", [128, 128])
    out = nc.dram_tensor("out", [S, D], f32, kind="ExternalOutput").ap()

    inv_sqrt_d = 1.0 / math.sqrt(D)

    with tile.TileContext(nc) as tc:
        with (
            tc.tile_pool(name="const", bufs=1) as cp,
            tc.tile_pool(name="wpool", bufs=5) as wp,
            tc.tile_pool(name="acts", bufs=1) as acp,
            tc.tile_pool(name="state", bufs=1) as sp,
            tc.tile_pool(name="expp", bufs=4) as ep,
            tc.tile_pool(name="zrp", bufs=2) as zp,
            tc.tile_pool(name="owp", bufs=4) as owp,
            tc.tile_pool(name="finp", bufs=2) as fp_,
            tc.tile_pool(name="psA", bufs=2, space="PSUM") as psA,
            tc.tile_pool(name="psO", bufs=4, space="PSUM") as psO,
            tc.tile_pool(name="psZ", bufs=2, space="PSUM") as psZ,
        ):
            # ---- constants / small inputs ----
            ones_sb = cp.tile([128, 128], b16, tag="ones")
            nc.sync.dma_start(out=ones_sb, in_=ones)
            wk_bT_sb = cp.tile([128, JT], f32, tag="wkb")
            nc.sync.dma_start(out=wk_bT_sb, in_=wk_bT)
            wq_bT_sb = cp.tile([128, JT], f32, tag="wqb")
            nc.sync.dma_start(out=wq_bT_sb, in_=wq_bT)
            wv_br_sb = cp.tile([128, HEADS * D], f32, tag="wvb")
            nc.sync.dma_start(out=wv_br_sb, in_=wv_br)
            out_br_sb = cp.tile([128, D], f32, tag="outb")
            nc.sync.dma_start(out=out_br_sb, in_=out_br)

            # ---- activations in (d on partitions, 4 d-tiles side by side) ----
            qT_sb = acp.tile([128, DT * S], b16, tag="qT")
            kT_sb = acp.tile([128, DT * SEQ], b16, tag="kT")
            vT_sb = acp.tile([128, DT * SEQ], b16, tag="vT")
            for dt_ in range(DT):
                nc.sync.dma_start(out=qT_sb[:, dt_ * S:(dt_ + 1) * S],
                                  in_=qT[dt_ * 128:(dt_ + 1) * 128, :])
                nc.sync.dma_start(out=kT_sb[:, dt_ * SEQ:(dt_ + 1) * SEQ],
                                  in_=kT[dt_ * 128:(dt_ + 1) * 128, :])
                nc.sync.dma_start(out=vT_sb[:, dt_ * SEQ:(dt_ + 1) * SEQ],
                                  in_=vT[dt_ * 128:(dt_ + 1) * 128, :])

            # ---- persistent state ----
            qpT_sb = sp.tile([128, JT * S], b16, tag="qpT")       # 16KB/part
            kpT_sb = sp.tile([128, JT * SEQ], b16, tag="kpT")     # 32KB/part
            vp_sb = sp.tile([128, TT * HEADS * D], b16, tag="vp")  # 32KB/part
            oT_sb = sp.tile([128, DT * 2048], b16, tag="oT")  # 16KB/part

            def load_w(dram):
                tiles = []
                for dt_ in range(DT):
                    t = wp.tile([128, HEADS * D], b16, tag="w")
                    nc.sync.dma_start(out=t, in_=dram[dt_ * 128:(dt_ + 1) * 128, :])
                    tiles.append(t)
                return tiles

            # ---- phase A1: qpT[j, s] = wkT.T @ qT + wk_b ----
            wk_sb = load_w(wkT)
            for jt in range(JT):
                ps = psA.tile([128, 512], f32, tag="psA")
                for dt_ in range(DT):
                    nc.tensor.matmul(
                        ps[:, :S],
                        lhsT=wk_sb[dt_][:, jt * 128:(jt + 1) * 128],
                        rhs=qT_sb[:, dt_ * S:(dt_ + 1) * S],
                        start=(dt_ == 0), stop=(dt_ == DT - 1))
                nc.scalar.activation(qpT_sb[:, jt * S:(jt + 1) * S], ps[:, :S],
                                     Act.Identity, bias=wk_bT_sb[:, jt:jt + 1],
                                     scale=1.0)

            # ---- phase A2: kpT[j, t] = wqT.T @ kT + wq_b ----
            wq_sb = load_w(wqT)
            for jt in range(JT):
                ps = psA.tile([128, 512], f32, tag="psA")
                for dt_ in range(DT):
                    nc.tensor.matmul(
                        ps,
                        lhsT=wq_sb[dt_][:, jt * 128:(jt + 1) * 128],
                        rhs=kT_sb[:, dt_ * SEQ:(dt_ + 1) * SEQ],
                        start=(dt_ == 0), stop=(dt_ == DT - 1))
                nc.scalar.activation(kpT_sb[:, jt * SEQ:(jt + 1) * SEQ], ps,
                                     Act.Identity, bias=wq_bT_sb[:, jt:jt + 1],
                                     scale=1.0)

            # ---- phase A3: vp[t, j] = vT.T @ wvT + wv_b (natural layout) ----
            wv_sb = load_w(wvT)
            for tt in range(TT):
                for js in range(HEADS):
                    ps = psA.tile([128, 512], f32, tag="psA")
                    for dt_ in range(DT):
                        nc.tensor.matmul(
                            ps,
                            lhsT=vT_sb[:, dt_ * SEQ + tt * 128:
                                       dt_ * SEQ + (tt + 1) * 128],
                            rhs=wv_sb[dt_][:, js * 512:(js + 1) * 512],
                            start=(dt_ == 0), stop=(dt_ == DT - 1))
                    nc.vector.tensor_add(
                        vp_sb[:, tt * HEADS * D + js * 512:
                              tt * HEADS * D + (js + 1) * 512],
                        ps, wv_br_sb[:, js * 512:(js + 1) * 512])

            # ---- phase B: attention, 4 l-slices of 512 query rows ----
            for ls in range(LSLICES):
                h0 = 2 * ls
                pz = psZ.tile([128, 512], f32, tag="psZ")
                po = [psO.tile([128, 512], f32, tag="psO", name=f"po{ls}_{i}")
                      for i in range(DT)]
                nchunk = HEADS * TT  # 32
                for g in range(HEADS):
                    for tt in range(TT):
                        ci = g * TT + tt
                        ps = psA.tile([128, 512], f32, tag="psA")
                        # scoresT[(g,t-tile), (h,s)] for h in (h0, h0+1)
                        for hi in range(2):
                            h = h0 + hi
                            for dt_ in range(DT):
                                nc.tensor.matmul(
                                    ps[:, hi * S:(hi + 1) * S],
                                    lhsT=kpT_sb[:, (g * DT + dt_) * SEQ + tt * 128:
                                                (g * DT + dt_) * SEQ + (tt + 1) * 128],
                                    rhs=qpT_sb[:, (h * DT + dt_) * S:
                                               (h * DT + dt_ + 1) * S],
                                    start=(dt_ == 0), stop=(dt_ == DT - 1))
                        ex = ep.tile([128, 512], b16, tag="exp")
                        nc.scalar.activation(ex, ps, Act.Exp, bias=0.0,
                                             scale=inv_sqrt_d)
                        # Z (col sums, replicated over partitions via ones)
                        nc.tensor.matmul(pz, lhsT=ones_sb, rhs=ex,
                                         start=(ci == 0), stop=(ci == nchunk - 1))
                        # unnormalized oT[e, l'] accumulation
                        for et in range(DT):
                            nc.tensor.matmul(
                                po[et],
                                lhsT=vp_sb[:, tt * HEADS * D + g * 512 + et * 128:
                                           tt * HEADS * D + g * 512 + (et + 1) * 128],
                                rhs=ex,
                                start=(ci == 0), stop=(ci == nchunk - 1))
                zr = zp.tile([128, 512], f32, tag="zr")
                nc.vector.reciprocal(zr, pz)
                for et in range(DT):
                    nc.vector.tensor_mul(
                        oT_sb[:, et * 2048 + ls * 512:et * 2048 + (ls + 1) * 512],
                        po[et], zr)

            # ---- phase C: out[s, r] = oT.T @ owT + out_b ----
            psc = [psA.tile([128, 512], f32, tag="psA", name=f"psc{i}")
                   for i in range(2)]
            for ct in range(JT):
                h, et = divmod(ct, DT)
                ow_sb = owp.tile([128, D], b16, tag="ow")
                nc.sync.dma_start(out=ow_sb, in_=owT[ct * 128:(ct + 1) * 128, :])
                for st in range(2):
                    nc.tensor.matmul(
                        psc[st],
                        lhsT=oT_sb[:, et * 2048 + h * S + st * 128:
                                   et * 2048 + h * S + (st + 1) * 128],
                        rhs=ow_sb,
                        start=(ct == 0), stop=(ct == JT - 1))
            for st in range(2):
                fin = fp_.tile([128, D], f32, tag="fin")
                nc.vector.tensor_add(fin, psc[st], out_br_sb)
                nc.sync.dma_start(out=out[st * 128:(st + 1) * 128, :], in_=fin)

    nc.compile()
    return nc


def _get_program():
    if "nc" not in _CACHE:
        _CACHE["nc"] = _build_program()
    return _CACHE["nc"]


def _prep_shared(inputs):
    bf = NP_BF16
    f32c = np.ascontiguousarray
    shared = {
        "wkT": f32c(np.asarray(inputs["wk_w"], np.float32).T).astype(bf),
        "wqT": f32c(np.asarray(inputs["wq_w"], np.float32).T).astype(bf),
        "wvT": f32c(np.asarray(inputs["wv_w"], np.float32).T).astype(bf),
        "owT": f32c(np.asarray(inputs["out_w"], np.float32).T).astype(bf),
        "wk_bT": f32c(np.asarray(inputs["wk_b"], np.float32).reshape(JT, 128).T),
        "wq_bT": f32c(np.asarray(inputs["wq_b"], np.float32).reshape(JT, 128).T),
        "wv_br": f32c(np.broadcast_to(
            np.asarray(inputs["wv_b"], np.float32)[None, :], (128, HEADS * D))),
        "out_br": f32c(np.broadcast_to(
            np.asarray(inputs["out_b"], np.float32)[None, :], (128, D))),
        "ones": np.ones((128, 128), bf),
    }
    return shared


def _make_in_maps(inputs):
    bf = NP_BF16
    shared = _prep_shared(inputs)
    q = np.asarray(inputs["q"], np.float32)
    k = np.asarray(inputs["k"], np.float32)
    v = np.asarray(inputs["v"], np.float32)
    in_maps = []
    for core in range(NCORES):
        b, half = divmod(core, 2)
        m = dict(shared)
        m["qT"] = np.ascontiguousarray(q[b, half * S:(half + 1) * S, :].T).astype(bf)
        m["kT"] = np.ascontiguousarray(k[b].T).astype(bf)
        m["vT"] = np.ascontiguousarray(v[b].T).astype(bf)
        in_maps.append(m)
    return in_maps


def kernel(**inputs):
    from concourse.bass_utils import run_bass_kernel_spmd

    nc = _get_program()
    in_maps = _make_in_maps(inputs)
    res = run_bass_kernel_spmd(nc, in_maps, core_ids=list(range(NCORES)))
    _CACHE["last_results"] = res
    out = np.empty((BS, SEQ, D), np.float32)
    for core in range(NCORES):
        b, half = divmod(core, 2)
        out[b, half * S:(half + 1) * S, :] = res.results[core]["out"]
    return out


if __name__ == "__main__":
    rng = np.random.default_rng(0)
    fake = {
        "q": rng.standard_normal((BS, SEQ, D), np.float32),
        "k": rng.standard_normal((BS, SEQ, D), np.float32),
        "v": rng.standard_normal((BS, SEQ, D), np.float32),
        "wq_w": rng.standard_normal((D * HEADS, D), np.float32) * 0.02,
        "wq_b": rng.standard_normal((D * HEADS,), np.float32) * 0.02,
        "wk_w": rng.standard_normal((D * HEADS, D), np.float32) * 0.02,
        "wk_b": rng.standard_normal((D * HEADS,), np.float32) * 0.02,
        "wv_w": rng.standard_normal((D * HEADS, D), np.float32) * 0.02,
        "wv_b": rng.standard_normal((D * HEADS,), np.float32) * 0.02,
        "out_w": rng.standard_normal((D, D * HEADS), np.float32) * 0.02,
        "out_b": rng.standard_normal((D,), np.float32) * 0.02,
    }
    o = kernel(**fake)
    print("kernel ran, out shape", o.shape, "std", o.std())


# revision 5
# speedup vs baseline: 1.1102x; 1.1102x over previous
"""Trainium2 Bass kernel for nn_MultiHeadAttention_48086453846410.

Reference computation (heads folded into the sequence axis, softmax over the
FULL L = seq*heads key axis):
    qp = (q @ wk_w.T + wk_b).reshape(bs, L, d)   # note swapped wk/wq, faithful
    kp = (k @ wq_w.T + wq_b).reshape(bs, L, d)
    vp = (v @ wv_w.T + wv_b).reshape(bs, L, d)
    scores = qp @ kp.T / sqrt(d); attn = softmax(scores, -1)
    o = (attn @ vp).reshape(bs, seq, d*heads)
    out = o @ out_w.T + out_b

Sharding: 8 cores = (batch b in 0..3) x (seq half). Each core owns 256 query
seq positions of one batch (2048 query rows l' = h*256+s). Softmax is over
keys, so query rows are independent -> no collectives.

On-device layout strategy (all matmuls bf16 inputs, fp32 PSUM accumulate):
 - host pre-transposes activations/weights so no on-device transposes at all
 - qpT (interleaved d-tile-major layout so score matmuls take two heads per
   N=512 moving operand) / kpT computed transposed (proj dim j on partitions)
 - vp computed in natural layout (t on partitions)
 - scores computed transposed: scoresT[m=(g,t), l'] -> softmax needs only
   exp (scores bounded: |s| < 2, so no max subtraction) and the denominator
   Z, computed by a ones-matmul (replicated across partitions for free);
   normalization is deferred to oT columns.
 - attn@v consumes exp tiles directly as the moving operand -> oT (e on
   partitions), which is exactly the lhsT layout for the out projection.
 - out projection runs per l-slice (pipelined into phase B) with fp32
   partial sums held in SBUF via DVE adds.
"""

import math
import sys

for _p in ("/opt/trn_rl_repo",):
    if _p not in sys.path:
        sys.path.insert(0, _p)

import numpy as np
import ml_dtypes

BS, SEQ, D, HEADS = 4, 512, 512, 8
NCORES = 8
S = SEQ // 2            # 256 query seq rows per core
JT = HEADS * D // 128   # 32 tiles of the 4096 projection dim
DT = D // 128           # 4 tiles of the 512 contraction dim
TT = SEQ // 128         # 4 key-seq tiles per head
LSLICES = 4             # l' = 2048 per core, processed in 4 slices of 512
WQCOLS = 1024           # weight streaming tile width (quarter tiles)
NP_BF16 = ml_dtypes.bfloat16

_CACHE = {}


def _build_program():
    from concourse import bacc
    import concourse.mybir as mybir
    import concourse.tile as tile
    from concourse.dt import dt

    f32 = dt.float32
    b16 = dt.bfloat16
    Act = mybir.ActivationFunctionType

    nc = bacc.Bacc(None, target_bir_lowering=False, debug=False,
                   num_devices=NCORES)

    def din(name, shape, dty=b16):
        return nc.dram_tensor(name, shape, dty, kind="ExternalInput").ap()

    qT = din("qT", [D, S])                 # q[b, half].T      (d, s)
    kT = din("kT", [D, SEQ])               # k[b].T            (d, t)
    vT = din("vT", [D, SEQ])               # v[b].T            (d, t)
    wkT = din("wkT", [D, HEADS * D])       # wk_w.T            (d, j)
    wqT = din("wqT", [D, HEADS * D])       # wq_w.T            (d, j)
    wvT = din("wvT", [D, HEADS * D])       # wv_w.T            (d, j)
    owT = din("owT", [HEADS * D, D])       # out_w.T           (c, r)
    wk_bT = din("wk_bT", [128, JT], f32)   # wk_b.reshape(JT,128).T
    wq_bT = din("wq_bT", [128, JT], f32)
    wv_br = din("wv_br", [128, HEADS * D], f32)   # wv_b replicated
    out_br = din("out_br", [128, D], f32)         # out_b replicated
    ones = din("ones", [128, 128])
    out = nc.dram_tensor("out", [S, D], f32, kind="ExternalOutput").ap()

    inv_sqrt_d = 1.0 / math.sqrt(D)
    NWQ = (HEADS * D) // WQCOLS  # 4 quarter-tiles per d-tile row

    with tile.TileContext(nc) as tc:
        with (
            tc.tile_pool(name="const", bufs=1) as cp,
            tc.tile_pool(name="wpool", bufs=20) as wp,
            tc.tile_pool(name="acts", bufs=1) as acp,
            tc.tile_pool(name="state", bufs=1) as sp,
            tc.tile_pool(name="expp", bufs=4) as ep,
            tc.tile_pool(name="zrp", bufs=2) as zp,
            tc.tile_pool(name="owp", bufs=8) as owp,
            tc.tile_pool(name="psA", bufs=2, space="PSUM") as psA,
            tc.tile_pool(name="psC", bufs=1, space="PSUM") as psC,
            tc.tile_pool(name="psO", bufs=4, space="PSUM") as psO,
            tc.tile_pool(name="psZ", bufs=1, space="PSUM") as psZ,
        ):
            # ---- weight streaming: quarter tiles (128 x WQCOLS) ----
            # tile index (dt, wq) covers d rows [dt*128,...), j cols
            # [wq*WQCOLS,...). Emission order = consumption order.
            def load_w(dram, nm):
                tiles = {}
                for dt_ in range(DT):
                    for wq in range(NWQ):
                        t = wp.tile([128, WQCOLS], b16, tag="w",
                                    name=f"w_{nm}_{dt_}_{wq}")
                        nc.sync.dma_start(
                            out=t,
                            in_=dram[dt_ * 128:(dt_ + 1) * 128,
                                     wq * WQCOLS:(wq + 1) * WQCOLS])
                        tiles[(dt_, wq)] = t
                return tiles

            def wslice(tiles, dt_, j0, width):
                wq, off = divmod(j0, WQCOLS)
                assert off + width <= WQCOLS
                return tiles[(dt_, wq)][:, off:off + width]

            # phase-A1 critical path first: wk weights + qT + its bias
            wk_sb = load_w(wkT, "k")
            qT_sb = acp.tile([128, DT * S], b16, tag="qT")
            for dt_ in range(DT):
                nc.sync.dma_start(out=qT_sb[:, dt_ * S:(dt_ + 1) * S],
                                  in_=qT[dt_ * 128:(dt_ + 1) * 128, :])
            wk_bT_sb = cp.tile([128, JT], f32, tag="wkb")
            nc.sync.dma_start(out=wk_bT_sb, in_=wk_bT)

            kT_sb = acp.tile([128, DT * SEQ], b16, tag="kT")
            vT_sb = acp.tile([128, DT * SEQ], b16, tag="vT")
            for dt_ in range(DT):
                nc.sync.dma_start(out=kT_sb[:, dt_ * SEQ:(dt_ + 1) * SEQ],
                                  in_=kT[dt_ * 128:(dt_ + 1) * 128, :])
            for dt_ in range(DT):
                nc.sync.dma_start(out=vT_sb[:, dt_ * SEQ:(dt_ + 1) * SEQ],
                                  in_=vT[dt_ * 128:(dt_ + 1) * 128, :])
            wq_bT_sb = cp.tile([128, JT], f32, tag="wqb")
            nc.sync.dma_start(out=wq_bT_sb, in_=wq_bT)
            ones_sb = cp.tile([128, 128], b16, tag="ones")
            nc.sync.dma_start(out=ones_sb, in_=ones)
            wv_br_sb = cp.tile([128, HEADS * D], f32, tag="wvb")
            nc.sync.dma_start(out=wv_br_sb, in_=wv_br)
            out_br_sb = cp.tile([128, D], f32, tag="outb")
            nc.sync.dma_start(out=out_br_sb, in_=out_br)

            # ---- persistent state ----
            # qpT interleaved: col block (dt*HEADS + h)*S
            qpT_sb = sp.tile([128, JT * S], b16, tag="qpT")       # 16KB/part
            kpT_sb = sp.tile([128, JT * SEQ], b16, tag="kpT")     # 32KB/part
            vp_sb = sp.tile([128, TT * HEADS * D], b16, tag="vp")  # 32KB/part
            oT_sb = sp.tile([128, DT * 2048], b16, tag="oT")      # 16KB/part
            fin32 = sp.tile([128, 2 * D], f32, tag="fin32")       # 4KB/part

            # ---- phase A1: qpT[j, s] = wkT.T @ qT + wk_b ----
            for jt in range(JT):
                h, dt_of_j = divmod(jt, DT)
                ps = psA.tile([128, 512], f32, tag="psA")
                for dt_ in range(DT):
                    nc.tensor.matmul(
                        ps[:, :S],
                        lhsT=wslice(wk_sb, dt_, jt * 128, 128),
                        rhs=qT_sb[:, dt_ * S:(dt_ + 1) * S],
                        start=(dt_ == 0), stop=(dt_ == DT - 1))
                nc.scalar.activation(
                    qpT_sb[:, (dt_of_j * HEADS + h) * S:
                           (dt_of_j * HEADS + h + 1) * S],
                    ps[:, :S], Act.Identity,
                    bias=wk_bT_sb[:, jt:jt + 1], scale=1.0)

            # ---- phase A2: kpT[j, t] = wqT.T @ kT + wq_b ----
            wq_sb = load_w(wqT, "q")
            for jt in range(JT):
                ps = psA.tile([128, 512], f32, tag="psA")
                for dt_ in range(DT):
                    nc.tensor.matmul(
                        ps,
                        lhsT=wslice(wq_sb, dt_, jt * 128, 128),
                        rhs=kT_sb[:, dt_ * SEQ:(dt_ + 1) * SEQ],
                        start=(dt_ == 0), stop=(dt_ == DT - 1))
                nc.scalar.activation(kpT_sb[:, jt * SEQ:(jt + 1) * SEQ], ps,
                                     Act.Identity, bias=wq_bT_sb[:, jt:jt + 1],
                                     scale=1.0)

            # ---- phase A3: vp[t, j] = vT.T @ wvT + wv_b (natural layout) ----
            wv_sb = load_w(wvT, "v")
            for tt in range(TT):
                for js in range(HEADS):
                    ps = psA.tile([128, 512], f32, tag="psA")
                    for dt_ in range(DT):
                        nc.tensor.matmul(
                            ps,
                            lhsT=vT_sb[:, dt_ * SEQ + tt * 128:
                                       dt_ * SEQ + (tt + 1) * 128],
                            rhs=wslice(wv_sb, dt_, js * 512, 512),
                            start=(dt_ == 0), stop=(dt_ == DT - 1))
                    nc.vector.tensor_add(
                        vp_sb[:, tt * HEADS * D + js * 512:
                              tt * HEADS * D + (js + 1) * 512],
                        ps, wv_br_sb[:, js * 512:(js + 1) * 512])

            # ---- phase B + pipelined out-projection, 4 l-slices ----
            for ls in range(LSLICES):
                h0 = 2 * ls
                pz = psZ.tile([128, 512], f32, tag="psZ", name=f"pz{ls}")
                po = [psO.tile([128, 512], f32, tag="psO", name=f"po{ls}_{i}")
                      for i in range(DT)]
                nchunk = HEADS * TT  # 32
                for g in range(HEADS):
                    for tt in range(TT):
                        ci = g * TT + tt
                        ps = psA.tile([128, 512], f32, tag="psA")
                        # scoresT[(g,tt), (h0..h0+1, s)] - both heads per MM
                        for dt_ in range(DT):
                            nc.tensor.matmul(
                                ps,
                                lhsT=kpT_sb[:, (g * DT + dt_) * SEQ + tt * 128:
                                            (g * DT + dt_) * SEQ + (tt + 1) * 128],
                                rhs=qpT_sb[:, (dt_ * HEADS + h0) * S:
                                           (dt_ * HEADS + h0 + 2) * S],
                                start=(dt_ == 0), stop=(dt_ == DT - 1))
                        ex = ep.tile([128, 512], b16, tag="exp")
                        nc.scalar.activation(ex, ps, Act.Exp, bias=0.0,
                                             scale=inv_sqrt_d)
                        # Z (col sums, replicated over partitions via ones)
                        nc.tensor.matmul(pz, lhsT=ones_sb, rhs=ex,
                                         start=(ci == 0), stop=(ci == nchunk - 1))
                        # unnormalized oT[e, l'] accumulation
                        for et in range(DT):
                            nc.tensor.matmul(
                                po[et],
                                lhsT=vp_sb[:, tt * HEADS * D + g * 512 + et * 128:
                                           tt * HEADS * D + g * 512 + (et + 1) * 128],
                                rhs=ex,
                                start=(ci == 0), stop=(ci == nchunk - 1))
                zr = zp.tile([128, 512], f32, tag="zr")
                nc.vector.reciprocal(zr, pz)
                for et in range(DT):
                    nc.vector.tensor_mul(
                        oT_sb[:, et * 2048 + ls * 512:et * 2048 + (ls + 1) * 512],
                        po[et], zr)

                # out-projection contribution of this l-slice:
                # c-tiles ct = h*DT+et for h in (h0, h0+1)
                ow_tiles = {}
                for st in range(2):
                    psc = psC.tile([128, 512], f32, tag="psC",
                                   name=f"psc{ls}_{st}")
                    for ci2, ct in enumerate(range(h0 * DT, (h0 + 2) * DT)):
                        h, et = divmod(ct, DT)
                        if st == 0:
                            ow_tiles[ct] = owp.tile([128, D], b16, tag="ow",
                                                    name=f"ow{ct}")
                            nc.sync.dma_start(
                                out=ow_tiles[ct],
                                in_=owT[ct * 128:(ct + 1) * 128, :])
                        nc.tensor.matmul(
                            psc,
                            lhsT=oT_sb[:, et * 2048 + h * S + st * 128:
                                       et * 2048 + h * S + (st + 1) * 128],
                            rhs=ow_tiles[ct],
                            start=(ci2 == 0), stop=(ci2 == 2 * DT - 1))
                    if ls == 0:
                        nc.vector.tensor_add(fin32[:, st * D:(st + 1) * D],
                                             psc, out_br_sb)
                    else:
                        nc.vector.tensor_add(fin32[:, st * D:(st + 1) * D],
                                             psc, fin32[:, st * D:(st + 1) * D])

            for st in range(2):
                nc.sync.dma_start(out=out[st * 128:(st + 1) * 128, :],
                                  in_=fin32[:, st * D:(st + 1) * D])

    nc.compile()
    return nc


def _get_program():
    if "nc" not in _CACHE:
        _CACHE["nc"] = _build_program()
    return _CACHE["nc"]


def _prep_shared(inputs):
    bf = NP_BF16
    f32c = np.ascontiguousarray
    shared = {
        "wkT": f32c(np.asarray(inputs["wk_w"], np.float32).T).astype(bf),
        "wqT": f32c(np.asarray(inputs["wq_w"], np.float32).T).astype(bf),
        "wvT": f32c(np.asarray(inputs["wv_w"], np.float32).T).astype(bf),
        "owT": f32c(np.asarray(inputs["out_w"], np.float32).T).astype(bf),
        "wk_bT": f32c(np.asarray(inputs["wk_b"], np.float32).reshape(JT, 128).T),
        "wq_bT": f32c(np.asarray(inputs["wq_b"], np.float32).reshape(JT, 128).T),
        "wv_br": f32c(np.broadcast_to(
            np.asarray(inputs["wv_b"], np.float32)[None, :], (128, HEADS * D))),
        "out_br": f32c(np.broadcast_to(
            np.asarray(inputs["out_b"], np.float32)[None, :], (128, D))),
        "ones": np.ones((128, 128), bf),
    }
    return shared


def _make_in_maps(inputs):
    bf = NP_BF16
    shared = _prep_shared(inputs)
    q = np.asarray(inputs["q"], np.float32)
    k = np.asarray(inputs["k"], np.float32)
    v = np.asarray(inputs["v"], np.float32)
    in_maps = []
    for core in range(NCORES):
        b, half = divmod(core, 2)
        m = dict(shared)
        m["qT"] = np.ascontiguousarray(q[b, half * S:(half + 1) * S, :].T).astype(bf)
        m["kT"] = np.ascontiguousarray(k[b].T).astype(bf)
        m["vT"] = np.ascontiguousarray(v[b].T).astype(bf)
        in_maps.append(m)
    return in_maps


def kernel(**inputs):
    from concourse.bass_utils import run_bass_kernel_spmd

    nc = _get_program()
    in_maps = _make_in_maps(inputs)
    res = run_bass_kernel_spmd(nc, in_maps, core_ids=list(range(NCORES)))
    _CACHE["last_results"] = res
    out = np.empty((BS, SEQ, D), np.float32)
    for core in range(NCORES):
        b, half = divmod(core, 2)
        out[b, half * S:(half + 1) * S, :] = res.results[core]["out"]
    return out


if __name__ == "__main__":
    rng = np.random.default_rng(0)
    fake = {
        "q": rng.standard_normal((BS, SEQ, D)).astype(np.float32),
        "k": rng.standard_normal((BS, SEQ, D)).astype(np.float32),
        "v": rng.standard_normal((BS, SEQ, D)).astype(np.float32),
        "wq_w": (rng.standard_normal((D * HEADS, D)) * 0.02).astype(np.float32),
        "wq_b": (rng.standard_normal((D * HEADS,)) * 0.02).astype(np.float32),
        "wk_w": (rng.standard_normal((D * HEADS, D)) * 0.02).astype(np.float32),
        "wk_b": (rng.standard_normal((D * HEADS,)) * 0.02).astype(np.float32),
        "wv_w": (rng.standard_normal((D * HEADS, D)) * 0.02).astype(np.float32),
        "wv_b": (rng.standard_normal((D * HEADS,)) * 0.02).astype(np.float32),
        "out_w": (rng.standard_normal((D, D * HEADS)) * 0.02).astype(np.float32),
        "out_b": (rng.standard_normal((D,)) * 0.02).astype(np.float32),
    }
    o = kernel(**fake)
    print("kernel ran, out shape", o.shape, "std", o.std())


# revision 9
# speedup vs baseline: 1.1303x; 1.0181x over previous
"""Trainium2 Bass kernel for nn_MultiHeadAttention_48086453846410.

Reference computation (heads folded into the sequence axis, softmax over the
FULL L = seq*heads key axis):
    qp = (q @ wk_w.T + wk_b).reshape(bs, L, d)   # note swapped wk/wq, faithful
    kp = (k @ wq_w.T + wq_b).reshape(bs, L, d)
    vp = (v @ wv_w.T + wv_b).reshape(bs, L, d)
    scores = qp @ kp.T / sqrt(d); attn = softmax(scores, -1)
    o = (attn @ vp).reshape(bs, seq, d*heads)
    out = o @ out_w.T + out_b

Sharding: 8 cores = (batch b in 0..3) x (seq half). Each core owns 256 query
seq positions of one batch (2048 query rows l' = h*256+s). Softmax is over
keys, so query rows are independent -> no collectives.

On-device layout strategy (all matmuls bf16 inputs, fp32 PSUM accumulate):
 - host pre-transposes activations/weights so no on-device transposes at all
 - qpT (interleaved d-tile-major layout so score matmuls take two heads per
   N=512 moving operand) / kpT computed transposed (proj dim j on partitions)
 - vp computed in natural layout (t on partitions)
 - scores computed transposed: scoresT[m=(g,t), l'] -> softmax needs only
   exp (scores bounded: |s| < 2, so no max subtraction) and the denominator
   Z, computed by a ones-matmul (replicated across partitions for free);
   normalization is deferred to oT columns.
 - attn@v consumes exp tiles directly as the moving operand -> oT (e on
   partitions), which is exactly the lhsT layout for the out projection.
 - out projection runs per l-slice (pipelined into phase B) with fp32
   partial sums held in SBUF via DVE adds.
"""

import math
import sys

for _p in ("/opt/trn_rl_repo",):
    if _p not in sys.path:
        sys.path.insert(0, _p)

import numpy as np
import ml_dtypes

BS, SEQ, D, HEADS = 4, 512, 512, 8
NCORES = 8
S = SEQ // 2            # 256 query seq rows per core
JT = HEADS * D // 128   # 32 tiles of the 4096 projection dim
DT = D // 128           # 4 tiles of the 512 contraction dim
TT = SEQ // 128         # 4 key-seq tiles per head
LSLICES = 4             # l' = 2048 per core, processed in 4 slices of 512
WQCOLS = 1024           # weight streaming tile width (quarter tiles)
NP_BF16 = ml_dtypes.bfloat16

_CACHE = {}


def _build_program():
    from concourse import bacc
    import concourse.mybir as mybir
    import concourse.tile as tile
    from concourse.dt import dt

    f32 = dt.float32
    b16 = dt.bfloat16
    Act = mybir.ActivationFunctionType

    nc = bacc.Bacc(None, target_bir_lowering=False, debug=False,
                   num_devices=NCORES)

    def din(name, shape, dty=b16):
        return nc.dram_tensor(name, shape, dty, kind="ExternalInput").ap()

    qT = din("qT", [D, S])                 # q[b, half].T      (d, s)
    kT = din("kT", [D, SEQ])               # k[b].T            (d, t)
    vT = din("vT", [D, SEQ])               # v[b].T            (d, t)
    wkT = din("wkT", [D, HEADS * D])       # wk_w.T            (d, j)
    wqT = din("wqT", [D, HEADS * D])       # wq_w.T            (d, j)
    wvT = din("wvT", [D, HEADS * D])       # wv_w.T            (d, j)
    owT = din("owT", [HEADS * D, D])       # out_w.T           (c, r)
    wk_bT = din("wk_bT", [128, JT], f32)   # wk_b.reshape(JT,128).T
    wq_bT = din("wq_bT", [128, JT], f32)
    wv_br = din("wv_br", [128, HEADS * D], f32)   # wv_b replicated
    out_br = din("out_br", [128, D], f32)         # out_b replicated
    ones = din("ones", [128, 128])
    out = nc.dram_tensor("out", [S, D], f32, kind="ExternalOutput").ap()

    inv_sqrt_d = 1.0 / math.sqrt(D)
    NWQ = (HEADS * D) // WQCOLS  # 4 quarter-tiles per d-tile row

    with tile.TileContext(nc) as tc:
        with (
            tc.tile_pool(name="const", bufs=1) as cp,
            tc.tile_pool(name="wpool", bufs=20) as wp,
            tc.tile_pool(name="acts", bufs=1) as acp,
            tc.tile_pool(name="state", bufs=1) as sp,
            tc.tile_pool(name="expp", bufs=4) as ep,
            tc.tile_pool(name="zrp", bufs=2) as zp,
            tc.tile_pool(name="owp", bufs=8) as owp,
            tc.tile_pool(name="psA", bufs=2, space="PSUM") as psA,
            tc.tile_pool(name="psC", bufs=1, space="PSUM") as psC,
            tc.tile_pool(name="psO", bufs=4, space="PSUM") as psO,
            tc.tile_pool(name="psZ", bufs=1, space="PSUM") as psZ,
        ):
            # ---- weight streaming: quarter tiles (128 x WQCOLS) ----
            # tile index (dt, wq) covers d rows [dt*128,...), j cols
            # [wq*WQCOLS,...). Emission order = consumption order.
            def load_w(dram, nm):
                tiles = {}
                for wq in range(NWQ):
                    for dt_ in range(DT):
                        t = wp.tile([128, WQCOLS], b16, tag="w",
                                    name=f"w_{nm}_{dt_}_{wq}")
                        nc.sync.dma_start(
                            out=t,
                            in_=dram[dt_ * 128:(dt_ + 1) * 128,
                                     wq * WQCOLS:(wq + 1) * WQCOLS])
                        tiles[(dt_, wq)] = t
                return tiles

            def wslice(tiles, dt_, j0, width):
                wq, off = divmod(j0, WQCOLS)
                assert off + width <= WQCOLS
                return tiles[(dt_, wq)][:, off:off + width]

            # phase-A1 critical path first: qT (small) then wk weights
            qT_sb = acp.tile([128, DT * S], b16, tag="qT")
            for dt_ in range(DT):
                nc.sync.dma_start(out=qT_sb[:, dt_ * S:(dt_ + 1) * S],
                                  in_=qT[dt_ * 128:(dt_ + 1) * 128, :])
            wk_bT_sb = cp.tile([128, JT], f32, tag="wkb")
            nc.sync.dma_start(out=wk_bT_sb, in_=wk_bT)
            wk_sb = load_w(wkT, "k")

            kT_sb = acp.tile([128, DT * SEQ], b16, tag="kT")
            vT_sb = acp.tile([128, DT * SEQ], b16, tag="vT")
            for dt_ in range(DT):
                nc.sync.dma_start(out=kT_sb[:, dt_ * SEQ:(dt_ + 1) * SEQ],
                                  in_=kT[dt_ * 128:(dt_ + 1) * 128, :])
            for dt_ in range(DT):
                nc.sync.dma_start(out=vT_sb[:, dt_ * SEQ:(dt_ + 1) * SEQ],
                                  in_=vT[dt_ * 128:(dt_ + 1) * 128, :])
            wq_bT_sb = cp.tile([128, JT], f32, tag="wqb")
            nc.sync.dma_start(out=wq_bT_sb, in_=wq_bT)
            ones_sb = cp.tile([128, 128], b16, tag="ones")
            nc.sync.dma_start(out=ones_sb, in_=ones)
            wv_br_sb = cp.tile([128, HEADS * D], f32, tag="wvb")
            nc.sync.dma_start(out=wv_br_sb, in_=wv_br)
            out_br_sb = cp.tile([128, D], f32, tag="outb")
            nc.sync.dma_start(out=out_br_sb, in_=out_br)

            # ---- persistent state ----
            # qpT interleaved: col block (dt*HEADS + h)*S
            qpT_sb = sp.tile([128, JT * S], b16, tag="qpT")       # 16KB/part
            kpT_sb = sp.tile([128, JT * SEQ], b16, tag="kpT")     # 32KB/part
            vp_sb = sp.tile([128, TT * HEADS * D], b16, tag="vp")  # 32KB/part
            oT_sb = sp.tile([128, DT * 2048], b16, tag="oT")      # 16KB/part
            fin32 = sp.tile([128, 2 * D], f32, tag="fin32")       # 4KB/part

            # ---- phase A1: qpT[j, s] = wkT.T @ qT + wk_b ----
            for jt in range(JT):
                h, dt_of_j = divmod(jt, DT)
                ps = psA.tile([128, 512], f32, tag="psA")
                for dt_ in range(DT):
                    nc.tensor.matmul(
                        ps[:, :S],
                        lhsT=wslice(wk_sb, dt_, jt * 128, 128),
                        rhs=qT_sb[:, dt_ * S:(dt_ + 1) * S],
                        start=(dt_ == 0), stop=(dt_ == DT - 1))
                nc.scalar.activation(
                    qpT_sb[:, (dt_of_j * HEADS + h) * S:
                           (dt_of_j * HEADS + h + 1) * S],
                    ps[:, :S], Act.Identity,
                    bias=wk_bT_sb[:, jt:jt + 1], scale=1.0)

            # ---- phase A2: kpT[j, t] = wqT.T @ kT + wq_b ----
            wq_sb = load_w(wqT, "q")
            for jt in range(JT):
                ps = psA.tile([128, 512], f32, tag="psA")
                for dt_ in range(DT):
                    nc.tensor.matmul(
                        ps,
                        lhsT=wslice(wq_sb, dt_, jt * 128, 128),
                        rhs=kT_sb[:, dt_ * SEQ:(dt_ + 1) * SEQ],
                        start=(dt_ == 0), stop=(dt_ == DT - 1))
                nc.scalar.activation(kpT_sb[:, jt * SEQ:(jt + 1) * SEQ], ps,
                                     Act.Identity, bias=wq_bT_sb[:, jt:jt + 1],
                                     scale=1.0)

            # ---- phase A3: vp[t, j] = vT.T @ wvT + wv_b (natural layout) ----
            wv_sb = load_w(wvT, "v")
            for tt in range(TT):
                for js in range(HEADS):
                    ps = psA.tile([128, 512], f32, tag="psA")
                    for dt_ in range(DT):
                        nc.tensor.matmul(
                            ps,
                            lhsT=vT_sb[:, dt_ * SEQ + tt * 128:
                                       dt_ * SEQ + (tt + 1) * 128],
                            rhs=wslice(wv_sb, dt_, js * 512, 512),
                            start=(dt_ == 0), stop=(dt_ == DT - 1))
                    nc.vector.tensor_add(
                        vp_sb[:, tt * HEADS * D + js * 512:
                              tt * HEADS * D + (js + 1) * 512],
                        ps, wv_br_sb[:, js * 512:(js + 1) * 512])

            # ---- phase B + pipelined out-projection, 4 l-slices ----
            for ls in range(LSLICES):
                h0 = 2 * ls
                pz = psZ.tile([128, 512], f32, tag="psZ", name=f"pz{ls}")
                po = [psO.tile([128, 512], f32, tag="psO", name=f"po{ls}_{i}")
                      for i in range(DT)]
                nchunk = HEADS * TT  # 32
                for g in range(HEADS):
                    for tt in range(TT):
                        ci = g * TT + tt
                        ps = psA.tile([128, 512], f32, tag="psA")
                        # scoresT[(g,tt), (h0..h0+1, s)] - both heads per MM
                        for dt_ in range(DT):
                            nc.tensor.matmul(
                                ps,
                                lhsT=kpT_sb[:, (g * DT + dt_) * SEQ + tt * 128:
                                            (g * DT + dt_) * SEQ + (tt + 1) * 128],
                                rhs=qpT_sb[:, (dt_ * HEADS + h0) * S:
                                           (dt_ * HEADS + h0 + 2) * S],
                                start=(dt_ == 0), stop=(dt_ == DT - 1))
                        ex = ep.tile([128, 512], b16, tag="exp")
                        nc.scalar.activation(ex, ps, Act.Exp, bias=0.0,
                                             scale=inv_sqrt_d)
                        # Z (col sums, replicated over partitions via ones)
                        nc.tensor.matmul(pz, lhsT=ones_sb, rhs=ex,
                                         start=(ci == 0), stop=(ci == nchunk - 1))
                        # unnormalized oT[e, l'] accumulation
                        for et in range(DT):
                            nc.tensor.matmul(
                                po[et],
                                lhsT=vp_sb[:, tt * HEADS * D + g * 512 + et * 128:
                                           tt * HEADS * D + g * 512 + (et + 1) * 128],
                                rhs=ex,
                                start=(ci == 0), stop=(ci == nchunk - 1))
                zr = zp.tile([128, 512], f32, tag="zr")
                nc.vector.reciprocal(zr, pz)
                for et in range(DT):
                    nc.vector.tensor_mul(
                        oT_sb[:, et * 2048 + ls * 512:et * 2048 + (ls + 1) * 512],
                        po[et], zr)

                # out-projection contribution of this l-slice:
                # c-tiles ct = h*DT+et for h in (h0, h0+1)
                ow_tiles = {}
                for st in range(2):
                    psc = psC.tile([128, 512], f32, tag="psC",
                                   name=f"psc{ls}_{st}")
                    for ci2, ct in enumerate(range(h0 * DT, (h0 + 2) * DT)):
                        h, et = divmod(ct, DT)
                        if st == 0:
                            ow_tiles[ct] = owp.tile([128, D], b16, tag="ow",
                                                    name=f"ow{ct}")
                            nc.sync.dma_start(
                                out=ow_tiles[ct],
                                in_=owT[ct * 128:(ct + 1) * 128, :])
                        nc.tensor.matmul(
                            psc,
                            lhsT=oT_sb[:, et * 2048 + h * S + st * 128:
                                       et * 2048 + h * S + (st + 1) * 128],
                            rhs=ow_tiles[ct],
                            start=(ci2 == 0), stop=(ci2 == 2 * DT - 1))
                    if ls == 0:
                        nc.vector.tensor_add(fin32[:, st * D:(st + 1) * D],
                                             psc, out_br_sb)
                    else:
                        nc.vector.tensor_add(fin32[:, st * D:(st + 1) * D],
                                             psc, fin32[:, st * D:(st + 1) * D])

            for st in range(2):
                nc.sync.dma_start(out=out[st * 128:(st + 1) * 128, :],
                                  in_=fin32[:, st * D:(st + 1) * D])

    nc.compile()
    return nc


def _get_program():
    if "nc" not in _CACHE:
        _CACHE["nc"] = _build_program()
    return _CACHE["nc"]


def _prep_shared(inputs):
    bf = NP_BF16
    f32c = np.ascontiguousarray
    shared = {
        "wkT": f32c(np.asarray(inputs["wk_w"], np.float32).T).astype(bf),
        "wqT": f32c(np.asarray(inputs["wq_w"], np.float32).T).astype(bf),
        "wvT": f32c(np.asarray(inputs["wv_w"], np.float32).T).astype(bf),
        "owT": f32c(np.asarray(inputs["out_w"], np.float32).T).astype(bf),
        "wk_bT": f32c(np.asarray(inputs["wk_b"], np.float32).reshape(JT, 128).T),
        "wq_bT": f32c(np.asarray(inputs["wq_b"], np.float32).reshape(JT, 128).T),
        "wv_br": f32c(np.broadcast_to(
            np.asarray(inputs["wv_b"], np.float32)[None, :], (128, HEADS * D))),
        "out_br": f32c(np.broadcast_to(
            np.asarray(inputs["out_b"], np.float32)[None, :], (128, D))),
        "ones": np.ones((128, 128), bf),
    }
    return shared


def _make_in_maps(inputs):
    bf = NP_BF16
    shared = _prep_shared(inputs)
    q = np.asarray(inputs["q"], np.float32)
    k = np.asarray(inputs["k"], np.float32)
    v = np.asarray(inputs["v"], np.float32)
    in_maps = []
    for core in range(NCORES):
        b, half = divmod(core, 2)
        m = dict(shared)
        m["qT"] = np.ascontiguousarray(q[b, half * S:(half + 1) * S, :].T).astype(bf)
        m["kT"] = np.ascontiguousarray(k[b].T).astype(bf)
        m["vT"] = np.ascontiguousarray(v[b].T).astype(bf)
        in_maps.append(m)
    return in_maps


def kernel(**inputs):
    from concourse.bass_utils import run_bass_kernel_spmd

    nc = _get_program()
    in_maps = _make_in_maps(inputs)
    res = run_bass_kernel_spmd(nc, in_maps, core_ids=list(range(NCORES)))
    _CACHE["last_results"] = res
    out = np.empty((BS, SEQ, D), np.float32)
    for core in range(NCORES):
        b, half = divmod(core, 2)
        out[b, half * S:(half + 1) * S, :] = res.results[core]["out"]
    return out


if __name__ == "__main__":
    rng = np.random.default_rng(0)
    fake = {
        "q": rng.standard_normal((BS, SEQ, D)).astype(np.float32),
        "k": rng.standard_normal((BS, SEQ, D)).astype(np.float32),
        "v": rng.standard_normal((BS, SEQ, D)).astype(np.float32),
        "wq_w": (rng.standard_normal((D * HEADS, D)) * 0.02).astype(np.float32),
        "wq_b": (rng.standard_normal((D * HEADS,)) * 0.02).astype(np.float32),
        "wk_w": (rng.standard_normal((D * HEADS, D)) * 0.02).astype(np.float32),
        "wk_b": (rng.standard_normal((D * HEADS,)) * 0.02).astype(np.float32),
        "wv_w": (rng.standard_normal((D * HEADS, D)) * 0.02).astype(np.float32),
        "wv_b": (rng.standard_normal((D * HEADS,)) * 0.02).astype(np.float32),
        "out_w": (rng.standard_normal((D, D * HEADS)) * 0.02).astype(np.float32),
        "out_b": (rng.standard_normal((D,)) * 0.02).astype(np.float32),
    }
    o = kernel(**fake)
    print("kernel ran, out shape", o.shape, "std", o.std())


# revision 12
# speedup vs baseline: 1.1306x; 1.0002x over previous
"""Trainium2 Bass kernel for nn_MultiHeadAttention_48086453846410.

Reference computation (heads folded into the sequence axis, softmax over the
FULL L = seq*heads key axis):
    qp = (q @ wk_w.T + wk_b).reshape(bs, L, d)   # note swapped wk/wq, faithful
    kp = (k @ wq_w.T + wq_b).reshape(bs, L, d)
    vp = (v @ wv_w.T + wv_b).reshape(bs, L, d)
    scores = qp @ kp.T / sqrt(d); attn = softmax(scores, -1)
    o = (attn @ vp).reshape(bs, seq, d*heads)
    out = o @ out_w.T + out_b

Sharding: 8 cores = (batch b in 0..3) x (seq half). Each core owns 256 query
seq positions of one batch (2048 query rows l' = h*256+s). Softmax is over
keys, so query rows are independent -> no collectives.

On-device layout strategy (all matmuls bf16 inputs, fp32 PSUM accumulate):
 - host pre-transposes activations/weights so no on-device transposes at all
 - qpT (interleaved d-tile-major layout so score matmuls take two heads per
   N=512 moving operand) / kpT computed transposed (proj dim j on partitions)
 - vp computed in natural layout (t on partitions)
 - scores computed transposed: scoresT[m=(g,t), l'] -> softmax needs only
   exp (scores bounded: |s| < 2, so no max subtraction) and the denominator
   Z, computed by a ones-matmul (replicated across partitions for free);
   normalization is deferred to oT columns.
 - attn@v consumes exp tiles directly as the moving operand -> oT (e on
   partitions), which is exactly the lhsT layout for the out projection.
 - out projection runs per l-slice (pipelined into phase B) with fp32
   partial sums held in SBUF via DVE adds.
"""

import math
import sys

for _p in ("/opt/trn_rl_repo",):
    if _p not in sys.path:
        sys.path.insert(0, _p)

import numpy as np
import ml_dtypes

BS, SEQ, D, HEADS = 4, 512, 512, 8
NCORES = 8
S = SEQ // 2            # 256 query seq rows per core
JT = HEADS * D // 128   # 32 tiles of the 4096 projection dim
DT = D // 128           # 4 tiles of the 512 contraction dim
TT = SEQ // 128         # 4 key-seq tiles per head
LSLICES = 4             # l' = 2048 per core, processed in 4 slices of 512
WQCOLS = 1024           # weight streaming tile width (quarter tiles)
NP_BF16 = ml_dtypes.bfloat16

_CACHE = {}


def _build_program():
    from concourse import bacc
    import concourse.mybir as mybir
    import concourse.tile as tile
    from concourse.dt import dt

    f32 = dt.float32
    b16 = dt.bfloat16
    Act = mybir.ActivationFunctionType

    nc = bacc.Bacc(None, target_bir_lowering=False, debug=False,
                   num_devices=NCORES)

    def din(name, shape, dty=b16):
        return nc.dram_tensor(name, shape, dty, kind="ExternalInput").ap()

    qT = din("qT", [D, S])                 # q[b, half].T      (d, s)
    kT = din("kT", [D, SEQ])               # k[b].T            (d, t)
    vT = din("vT", [D, SEQ])               # v[b].T            (d, t)
    wkT = din("wkT", [D, HEADS * D])       # wk_w.T            (d, j)
    wqT = din("wqT", [D, HEADS * D])       # wq_w.T            (d, j)
    wvT = din("wvT", [D, HEADS * D])       # wv_w.T            (d, j)
    owT = din("owT", [HEADS * D, D])       # out_w.T           (c, r)
    wk_bT = din("wk_bT", [128, JT], f32)   # wk_b.reshape(JT,128).T
    wq_bT = din("wq_bT", [128, JT], f32)
    wv_br = din("wv_br", [128, HEADS * D], f32)   # wv_b replicated
    out_br = din("out_br", [128, D], f32)         # out_b replicated
    ones = din("ones", [128, 128])
    out = nc.dram_tensor("out", [S, D], f32, kind="ExternalOutput").ap()

    inv_sqrt_d = 1.0 / math.sqrt(D)
    NWQ = (HEADS * D) // WQCOLS  # 4 quarter-tiles per d-tile row

    with tile.TileContext(nc) as tc:
        with (
            tc.tile_pool(name="const", bufs=1) as cp,
            tc.tile_pool(name="wpool", bufs=20) as wp,
            tc.tile_pool(name="acts", bufs=1) as acp,
            tc.tile_pool(name="state", bufs=1) as sp,
            tc.tile_pool(name="expp", bufs=6) as ep,
            tc.tile_pool(name="zrp", bufs=2) as zp,
            tc.tile_pool(name="owp", bufs=8) as owp,
            tc.tile_pool(name="psA", bufs=2, space="PSUM") as psA,
            tc.tile_pool(name="psC", bufs=1, space="PSUM") as psC,
            tc.tile_pool(name="psO", bufs=4, space="PSUM") as psO,
            tc.tile_pool(name="psZ", bufs=1, space="PSUM") as psZ,
        ):
            # ---- weight streaming: quarter tiles (128 x WQCOLS) ----
            # tile index (dt, wq) covers d rows [dt*128,...), j cols
            # [wq*WQCOLS,...). Emission order = consumption order.
            def load_w(dram, nm):
                tiles = {}
                for wq in range(NWQ):
                    for dt_ in range(DT):
                        t = wp.tile([128, WQCOLS], b16, tag="w",
                                    name=f"w_{nm}_{dt_}_{wq}")
                        nc.sync.dma_start(
                            out=t,
                            in_=dram[dt_ * 128:(dt_ + 1) * 128,
                                     wq * WQCOLS:(wq + 1) * WQCOLS])
                        tiles[(dt_, wq)] = t
                return tiles

            def wslice(tiles, dt_, j0, width):
                wq, off = divmod(j0, WQCOLS)
                assert off + width <= WQCOLS
                return tiles[(dt_, wq)][:, off:off + width]

            # phase-A1 critical path first: qT (small) then wk weights
            qT_sb = acp.tile([128, DT * S], b16, tag="qT")
            for dt_ in range(DT):
                nc.sync.dma_start(out=qT_sb[:, dt_ * S:(dt_ + 1) * S],
                                  in_=qT[dt_ * 128:(dt_ + 1) * 128, :])
            wk_bT_sb = cp.tile([128, JT], f32, tag="wkb")
            nc.sync.dma_start(out=wk_bT_sb, in_=wk_bT)
            wk_sb = load_w(wkT, "k")

            kT_sb = acp.tile([128, DT * SEQ], b16, tag="kT")
            vT_sb = acp.tile([128, DT * SEQ], b16, tag="vT")
            for dt_ in range(DT):
                nc.sync.dma_start(out=kT_sb[:, dt_ * SEQ:(dt_ + 1) * SEQ],
                                  in_=kT[dt_ * 128:(dt_ + 1) * 128, :])
            for dt_ in range(DT):
                nc.sync.dma_start(out=vT_sb[:, dt_ * SEQ:(dt_ + 1) * SEQ],
                                  in_=vT[dt_ * 128:(dt_ + 1) * 128, :])
            wq_bT_sb = cp.tile([128, JT], f32, tag="wqb")
            nc.sync.dma_start(out=wq_bT_sb, in_=wq_bT)
            ones_sb = cp.tile([128, 128], b16, tag="ones")
            nc.sync.dma_start(out=ones_sb, in_=ones)
            wv_br_sb = cp.tile([128, HEADS * D], f32, tag="wvb")
            nc.sync.dma_start(out=wv_br_sb, in_=wv_br)
            out_br_sb = cp.tile([128, D], f32, tag="outb")
            nc.sync.dma_start(out=out_br_sb, in_=out_br)

            # ---- persistent state ----
            # qpT interleaved: col block (dt*HEADS + h)*S
            qpT_sb = sp.tile([128, JT * S], b16, tag="qpT")       # 16KB/part
            kpT_sb = sp.tile([128, JT * SEQ], b16, tag="kpT")     # 32KB/part
            vp_sb = sp.tile([128, TT * HEADS * D], b16, tag="vp")  # 32KB/part
            oT_sb = sp.tile([128, DT * 2048], b16, tag="oT")      # 16KB/part
            fin32 = sp.tile([128, 2 * D], f32, tag="fin32")       # 4KB/part

            # ---- phase A1: qpT[j, s] = wkT.T @ qT + wk_b ----
            for jt in range(JT):
                h, dt_of_j = divmod(jt, DT)
                ps = psA.tile([128, 512], f32, tag="psA")
                for dt_ in range(DT):
                    nc.tensor.matmul(
                        ps[:, :S],
                        lhsT=wslice(wk_sb, dt_, jt * 128, 128),
                        rhs=qT_sb[:, dt_ * S:(dt_ + 1) * S],
                        start=(dt_ == 0), stop=(dt_ == DT - 1))
                nc.scalar.activation(
                    qpT_sb[:, (dt_of_j * HEADS + h) * S:
                           (dt_of_j * HEADS + h + 1) * S],
                    ps[:, :S], Act.Identity,
                    bias=wk_bT_sb[:, jt:jt + 1], scale=1.0)

            # ---- phase A2: kpT[j, t] = wqT.T @ kT + wq_b ----
            wq_sb = load_w(wqT, "q")
            for jt in range(JT):
                ps = psA.tile([128, 512], f32, tag="psA")
                for dt_ in range(DT):
                    nc.tensor.matmul(
                        ps,
                        lhsT=wslice(wq_sb, dt_, jt * 128, 128),
                        rhs=kT_sb[:, dt_ * SEQ:(dt_ + 1) * SEQ],
                        start=(dt_ == 0), stop=(dt_ == DT - 1))
                nc.scalar.activation(kpT_sb[:, jt * SEQ:(jt + 1) * SEQ], ps,
                                     Act.Identity, bias=wq_bT_sb[:, jt:jt + 1],
                                     scale=1.0)

            # ---- phase A3: vp[t, j] = vT.T @ wvT + wv_b (natural layout) ----
            wv_sb = load_w(wvT, "v")
            for tt in range(TT):
                for js in range(HEADS):
                    ps = psA.tile([128, 512], f32, tag="psA")
                    for dt_ in range(DT):
                        nc.tensor.matmul(
                            ps,
                            lhsT=vT_sb[:, dt_ * SEQ + tt * 128:
                                       dt_ * SEQ + (tt + 1) * 128],
                            rhs=wslice(wv_sb, dt_, js * 512, 512),
                            start=(dt_ == 0), stop=(dt_ == DT - 1))
                    nc.vector.tensor_add(
                        vp_sb[:, tt * HEADS * D + js * 512:
                              tt * HEADS * D + (js + 1) * 512],
                        ps, wv_br_sb[:, js * 512:(js + 1) * 512])

            # ---- phase B + pipelined out-projection, 4 l-slices ----
            for ls in range(LSLICES):
                h0 = 2 * ls
                pz = psZ.tile([128, 512], f32, tag="psZ", name=f"pz{ls}")
                po = [psO.tile([128, 512], f32, tag="psO", name=f"po{ls}_{i}")
                      for i in range(DT)]
                nchunk = HEADS * TT  # 32
                for g in range(HEADS):
                    for tt in range(TT):
                        ci = g * TT + tt
                        ps = psA.tile([128, 512], f32, tag="psA")
                        # scoresT[(g,tt), (h0..h0+1, s)] - both heads per MM
                        for dt_ in range(DT):
                            nc.tensor.matmul(
                                ps,
                                lhsT=kpT_sb[:, (g * DT + dt_) * SEQ + tt * 128:
                                            (g * DT + dt_) * SEQ + (tt + 1) * 128],
                                rhs=qpT_sb[:, (dt_ * HEADS + h0) * S:
                                           (dt_ * HEADS + h0 + 2) * S],
                                start=(dt_ == 0), stop=(dt_ == DT - 1))
                        ex = ep.tile([128, 512], b16, tag="exp")
                        nc.scalar.activation(ex, ps, Act.Exp, bias=0.0,
                                             scale=inv_sqrt_d)
                        # Z (col sums, replicated over partitions via ones)
                        nc.tensor.matmul(pz, lhsT=ones_sb, rhs=ex,
                                         start=(ci == 0), stop=(ci == nchunk - 1))
                        # unnormalized oT[e, l'] accumulation
                        for et in range(DT):
                            nc.tensor.matmul(
                                po[et],
                                lhsT=vp_sb[:, tt * HEADS * D + g * 512 + et * 128:
                                           tt * HEADS * D + g * 512 + (et + 1) * 128],
                                rhs=ex,
                                start=(ci == 0), stop=(ci == nchunk - 1))
                zr = zp.tile([128, 512], f32, tag="zr")
                nc.vector.reciprocal(zr, pz)
                for et in range(DT):
                    nc.vector.tensor_mul(
                        oT_sb[:, et * 2048 + ls * 512:et * 2048 + (ls + 1) * 512],
                        po[et], zr)

                # out-projection contribution of this l-slice:
                # c-tiles ct = h*DT+et for h in (h0, h0+1)
                ow_tiles = {}
                for st in range(2):
                    psc = psC.tile([128, 512], f32, tag="psC",
                                   name=f"psc{ls}_{st}")
                    for ci2, ct in enumerate(range(h0 * DT, (h0 + 2) * DT)):
                        h, et = divmod(ct, DT)
                        if st == 0:
                            ow_tiles[ct] = owp.tile([128, D], b16, tag="ow",
                                                    name=f"ow{ct}")
                            nc.sync.dma_start(
                                out=ow_tiles[ct],
                                in_=owT[ct * 128:(ct + 1) * 128, :])
                        nc.tensor.matmul(
                            psc,
                            lhsT=oT_sb[:, et * 2048 + h * S + st * 128:
                                       et * 2048 + h * S + (st + 1) * 128],
                            rhs=ow_tiles[ct],
                            start=(ci2 == 0), stop=(ci2 == 2 * DT - 1))
                    if ls == 0:
                        nc.vector.tensor_add(fin32[:, st * D:(st + 1) * D],
                                             psc, out_br_sb)
                    else:
                        nc.vector.tensor_add(fin32[:, st * D:(st + 1) * D],
                                             psc, fin32[:, st * D:(st + 1) * D])

            for st in range(2):
                nc.sync.dma_start(out=out[st * 128:(st + 1) * 128, :],
                                  in_=fin32[:, st * D:(st + 1) * D])

    nc.compile()
    return nc


def _get_program():
    if "nc" not in _CACHE:
        _CACHE["nc"] = _build_program()
    return _CACHE["nc"]


def _prep_shared(inputs):
    bf = NP_BF16
    f32c = np.ascontiguousarray
    shared = {
        "wkT": f32c(np.asarray(inputs["wk_w"], np.float32).T).astype(bf),
        "wqT": f32c(np.asarray(inputs["wq_w"], np.float32).T).astype(bf),
        "wvT": f32c(np.asarray(inputs["wv_w"], np.float32).T).astype(bf),
        "owT": f32c(np.asarray(inputs["out_w"], np.float32).T).astype(bf),
        "wk_bT": f32c(np.asarray(inputs["wk_b"], np.float32).reshape(JT, 128).T),
        "wq_bT": f32c(np.asarray(inputs["wq_b"], np.float32).reshape(JT, 128).T),
        "wv_br": f32c(np.broadcast_to(
            np.asarray(inputs["wv_b"], np.float32)[None, :], (128, HEADS * D))),
        "out_br": f32c(np.broadcast_to(
            np.asarray(inputs["out_b"], np.float32)[None, :], (128, D))),
        "ones": np.ones((128, 128), bf),
    }
    return shared


def _make_in_maps(inputs):
    bf = NP_BF16
    shared = _prep_shared(inputs)
    q = np.asarray(inputs["q"], np.float32)
    k = np.asarray(inputs["k"], np.float32)
    v = np.asarray(inputs["v"], np.float32)
    in_maps = []
    for core in range(NCORES):
        b, half = divmod(core, 2)
        m = dict(shared)
        m["qT"] = np.ascontiguousarray(q[b, half * S:(half + 1) * S, :].T).astype(bf)
        m["kT"] = np.ascontiguousarray(k[b].T).astype(bf)
        m["vT"] = np.ascontiguousarray(v[b].T).astype(bf)
        in_maps.append(m)
    return in_maps


def kernel(**inputs):
    from concourse.bass_utils import run_bass_kernel_spmd

    nc = _get_program()
    in_maps = _make_in_maps(inputs)
    res = run_bass_kernel_spmd(nc, in_maps, core_ids=list(range(NCORES)))
    _CACHE["last_results"] = res
    out = np.empty((BS, SEQ, D), np.float32)
    for core in range(NCORES):
        b, half = divmod(core, 2)
        out[b, half * S:(half + 1) * S, :] = res.results[core]["out"]
    return out


if __name__ == "__main__":
    rng = np.random.default_rng(0)
    fake = {
        "q": rng.standard_normal((BS, SEQ, D)).astype(np.float32),
        "k": rng.standard_normal((BS, SEQ, D)).astype(np.float32),
        "v": rng.standard_normal((BS, SEQ, D)).astype(np.float32),
        "wq_w": (rng.standard_normal((D * HEADS, D)) * 0.02).astype(np.float32),
        "wq_b": (rng.standard_normal((D * HEADS,)) * 0.02).astype(np.float32),
        "wk_w": (rng.standard_normal((D * HEADS, D)) * 0.02).astype(np.float32),
        "wk_b": (rng.standard_normal((D * HEADS,)) * 0.02).astype(np.float32),
        "wv_w": (rng.standard_normal((D * HEADS, D)) * 0.02).astype(np.float32),
        "wv_b": (rng.standard_normal((D * HEADS,)) * 0.02).astype(np.float32),
        "out_w": (rng.standard_normal((D, D * HEADS)) * 0.02).astype(np.float32),
        "out_b": (rng.standard_normal((D,)) * 0.02).astype(np.float32),
    }
    o = kernel(**fake)
    print("kernel ran, out shape", o.shape, "std", o.std())


# revision 20
# speedup vs baseline: 1.1437x; 1.0116x over previous
"""Trainium2 Bass kernel for nn_MultiHeadAttention_48086453846410.

Reference computation (heads folded into the sequence axis, softmax over the
FULL L = seq*heads key axis):
    qp = (q @ wk_w.T + wk_b).reshape(bs, L, d)   # note swapped wk/wq, faithful
    kp = (k @ wq_w.T + wq_b).reshape(bs, L, d)
    vp = (v @ wv_w.T + wv_b).reshape(bs, L, d)
    scores = qp @ kp.T / sqrt(d); attn = softmax(scores, -1)
    o = (attn @ vp).reshape(bs, seq, d*heads)
    out = o @ out_w.T + out_b

Sharding: 8 cores = (batch b in 0..3) x (seq half). Each core owns 256 query
seq positions of one batch (2048 query rows l' = h*256+s). Softmax is over
keys, so query rows are independent -> no collectives.

On-device layout strategy (all matmuls bf16 inputs, fp32 PSUM accumulate):
 - host pre-transposes activations/weights so no on-device transposes at all
 - qpT (interleaved d-tile-major layout so score matmuls take two heads per
   N=512 moving operand) / kpT computed transposed (proj dim j on partitions)
 - vp computed in natural layout (t on partitions)
 - scores computed transposed: scoresT[m=(g,t), l'] -> softmax needs only
   exp (scores bounded: |s| < 2, so no max subtraction) and the denominator
   Z, computed by a ones-matmul (replicated across partitions for free);
   normalization is deferred to oT columns.
 - attn@v consumes exp tiles directly as the moving operand -> oT (e on
   partitions), which is exactly the lhsT layout for the out projection.
 - out projection runs per l-slice (pipelined into phase B) with fp32
   partial sums held in SBUF via DVE adds.
"""

import math
import sys

for _p in ("/opt/trn_rl_repo",):
    if _p not in sys.path:
        sys.path.insert(0, _p)

import numpy as np
import ml_dtypes

BS, SEQ, D, HEADS = 4, 512, 512, 8
NCORES = 8
S = SEQ // 2            # 256 query seq rows per core
JT = HEADS * D // 128   # 32 tiles of the 4096 projection dim
DT = D // 128           # 4 tiles of the 512 contraction dim
TT = SEQ // 128         # 4 key-seq tiles per head
LSLICES = 4             # l' = 2048 per core, processed in 4 slices of 512
WQCOLS = 1024           # weight streaming tile width (quarter tiles)
NP_BF16 = ml_dtypes.bfloat16

_CACHE = {}


def _build_program():
    from concourse import bacc
    import concourse.mybir as mybir
    import concourse.tile as tile
    from concourse.dt import dt

    f32 = dt.float32
    b16 = dt.bfloat16
    Act = mybir.ActivationFunctionType

    nc = bacc.Bacc(None, target_bir_lowering=False, debug=False,
                   num_devices=NCORES)

    def din(name, shape, dty=b16):
        return nc.dram_tensor(name, shape, dty, kind="ExternalInput").ap()

    qT = din("qT", [D, S])                 # q[b, half].T      (d, s)
    kT = din("kT", [D, SEQ])               # k[b].T            (d, t)
    vT = din("vT", [D, SEQ])               # v[b].T            (d, t)
    wkT = din("wkT", [D, HEADS * D])       # wk_w.T            (d, j)
    wqT = din("wqT", [D, HEADS * D])       # wq_w.T            (d, j)
    wvT = din("wvT", [D, HEADS * D])       # wv_w.T            (d, j)
    owT = din("owT", [HEADS * D, D])       # out_w.T           (c, r)
    wk_bT = din("wk_bT", [128, JT], f32)   # wk_b.reshape(JT,128).T
    wq_bT = din("wq_bT", [128, JT], f32)
    wv_br = din("wv_br", [128, HEADS * D], f32)   # wv_b replicated
    out_br = din("out_br", [128, D], f32)         # out_b replicated
    ones = din("ones", [128, 128])
    out = nc.dram_tensor("out", [S, D], f32, kind="ExternalOutput").ap()

    inv_sqrt_d = 1.0 / math.sqrt(D)
    NWQ = (HEADS * D) // WQCOLS  # 4 quarter-tiles per d-tile row

    with tile.TileContext(nc) as tc:
        with (
            tc.tile_pool(name="const", bufs=1) as cp,
            tc.tile_pool(name="wpool", bufs=20) as wp,
            tc.tile_pool(name="acts", bufs=1) as acp,
            tc.tile_pool(name="state", bufs=1) as sp,
            tc.tile_pool(name="expp", bufs=6) as ep,
            tc.tile_pool(name="zrp", bufs=2) as zp,
            tc.tile_pool(name="owp", bufs=8) as owp,
            tc.tile_pool(name="psA", bufs=2, space="PSUM") as psA,
            tc.tile_pool(name="psC", bufs=1, space="PSUM") as psC,
            tc.tile_pool(name="psO", bufs=4, space="PSUM") as psO,
            tc.tile_pool(name="psZ", bufs=1, space="PSUM") as psZ,
        ):
            # ---- weight streaming: quarter tiles (128 x WQCOLS) ----
            # tile index (dt, wq) covers d rows [dt*128,...), j cols
            # [wq*WQCOLS,...). Emission order = consumption order.
            def load_w(dram, nm):
                tiles = {}
                for wq in range(NWQ):
                    for dt_ in range(DT):
                        t = wp.tile([128, WQCOLS], b16, tag="w",
                                    name=f"w_{nm}_{dt_}_{wq}")
                        eng = nc.sync if dt_ % 2 == 0 else nc.gpsimd
                        eng.dma_start(
                            out=t,
                            in_=dram[dt_ * 128:(dt_ + 1) * 128,
                                     wq * WQCOLS:(wq + 1) * WQCOLS])
                        tiles[(dt_, wq)] = t
                return tiles

            def wslice(tiles, dt_, j0, width):
                wq, off = divmod(j0, WQCOLS)
                assert off + width <= WQCOLS
                return tiles[(dt_, wq)][:, off:off + width]

            # phase-A1 critical path first: qT (small) then wk weights
            qT_sb = acp.tile([128, DT * S], b16, tag="qT")
            nc.sync.dma_start(out=qT_sb.rearrange("p (t n) -> p t n", n=S),
                              in_=qT.rearrange("(t p) n -> p t n", p=128))
            wk_bT_sb = cp.tile([128, JT], f32, tag="wkb")
            nc.sync.dma_start(out=wk_bT_sb, in_=wk_bT)
            wk_sb = load_w(wkT, "k")

            kT_sb = acp.tile([128, DT * SEQ], b16, tag="kT")
            vT_sb = acp.tile([128, DT * SEQ], b16, tag="vT")
            nc.sync.dma_start(out=kT_sb.rearrange("p (t n) -> p t n", n=SEQ),
                              in_=kT.rearrange("(t p) n -> p t n", p=128))
            nc.gpsimd.dma_start(out=vT_sb.rearrange("p (t n) -> p t n", n=SEQ),
                                in_=vT.rearrange("(t p) n -> p t n", p=128))
            wq_bT_sb = cp.tile([128, JT], f32, tag="wqb")
            nc.sync.dma_start(out=wq_bT_sb, in_=wq_bT)

            # ---- persistent state ----
            # qpT interleaved: col block (dt*HEADS + h)*S
            qpT_sb = sp.tile([128, JT * S], b16, tag="qpT")       # 16KB/part
            kpT_sb = sp.tile([128, JT * SEQ], b16, tag="kpT")     # 32KB/part
            vp_sb = sp.tile([128, TT * HEADS * D], b16, tag="vp")  # 32KB/part
            oT_sb = sp.tile([128, DT * 2048], b16, tag="oT")      # 16KB/part
            fin32 = sp.tile([128, 2 * D], f32, tag="fin32")       # 4KB/part

            # ---- phase A1: qpT[j, s] = wkT.T @ qT + wk_b ----
            for jt in range(JT):
                h, dt_of_j = divmod(jt, DT)
                ps = psA.tile([128, 512], f32, tag="psA")
                for dt_ in range(DT):
                    nc.tensor.matmul(
                        ps[:, :S],
                        lhsT=wslice(wk_sb, dt_, jt * 128, 128),
                        rhs=qT_sb[:, dt_ * S:(dt_ + 1) * S],
                        start=(dt_ == 0), stop=(dt_ == DT - 1))
                nc.scalar.activation(
                    qpT_sb[:, (dt_of_j * HEADS + h) * S:
                           (dt_of_j * HEADS + h + 1) * S],
                    ps[:, :S], Act.Identity,
                    bias=wk_bT_sb[:, jt:jt + 1], scale=1.0)

            # ---- phase A2: kpT[j, t] = wqT.T @ kT + wq_b ----
            wq_sb = load_w(wqT, "q")
            for jt in range(JT):
                ps = psA.tile([128, 512], f32, tag="psA")
                for dt_ in range(DT):
                    nc.tensor.matmul(
                        ps,
                        lhsT=wslice(wq_sb, dt_, jt * 128, 128),
                        rhs=kT_sb[:, dt_ * SEQ:(dt_ + 1) * SEQ],
                        start=(dt_ == 0), stop=(dt_ == DT - 1))
                nc.scalar.activation(kpT_sb[:, jt * SEQ:(jt + 1) * SEQ], ps,
                                     Act.Identity, bias=wq_bT_sb[:, jt:jt + 1],
                                     scale=1.0)

            # ---- phase A3: vp[t, j] = vT.T @ wvT + wv_b (natural layout) ----
            wv_sb = load_w(wvT, "v")
            wv_br_sb = cp.tile([128, HEADS * D], f32, tag="wvb")
            nc.sync.dma_start(out=wv_br_sb, in_=wv_br)
            ones_sb = cp.tile([128, 128], b16, tag="ones")
            nc.sync.dma_start(out=ones_sb, in_=ones)
            out_br_sb = cp.tile([128, D], f32, tag="outb")
            nc.sync.dma_start(out=out_br_sb, in_=out_br)
            for tt in range(TT):
                for js in range(HEADS):
                    ps = psA.tile([128, 512], f32, tag="psA")
                    for dt_ in range(DT):
                        nc.tensor.matmul(
                            ps,
                            lhsT=vT_sb[:, dt_ * SEQ + tt * 128:
                                       dt_ * SEQ + (tt + 1) * 128],
                            rhs=wslice(wv_sb, dt_, js * 512, 512),
                            start=(dt_ == 0), stop=(dt_ == DT - 1))
                    nc.vector.tensor_add(
                        vp_sb[:, tt * HEADS * D + js * 512:
                              tt * HEADS * D + (js + 1) * 512],
                        ps, wv_br_sb[:, js * 512:(js + 1) * 512])

            # ---- phase B + pipelined out-projection, 4 l-slices ----
            for ls in range(LSLICES):
                h0 = 2 * ls
                pz = psZ.tile([128, 512], f32, tag="psZ", name=f"pz{ls}")
                po = [psO.tile([128, 512], f32, tag="psO", name=f"po{ls}_{i}")
                      for i in range(DT)]
                nchunk = HEADS * TT  # 32
                for g in range(HEADS):
                    for tt in range(TT):
                        ci = g * TT + tt
                        ps = psA.tile([128, 512], f32, tag="psA")
                        # scoresT[(g,tt), (h0..h0+1, s)] - both heads per MM
                        for dt_ in range(DT):
                            nc.tensor.matmul(
                                ps,
                                lhsT=kpT_sb[:, (g * DT + dt_) * SEQ + tt * 128:
                                            (g * DT + dt_) * SEQ + (tt + 1) * 128],
                                rhs=qpT_sb[:, (dt_ * HEADS + h0) * S:
                                           (dt_ * HEADS + h0 + 2) * S],
                                start=(dt_ == 0), stop=(dt_ == DT - 1))
                        ex = ep.tile([128, 512], b16, tag="exp")
                        nc.scalar.activation(ex, ps, Act.Exp, bias=0.0,
                                             scale=inv_sqrt_d)
                        # Z (col sums, replicated over partitions via ones)
                        nc.tensor.matmul(pz, lhsT=ones_sb, rhs=ex,
                                         start=(ci == 0), stop=(ci == nchunk - 1))
                        # unnormalized oT[e, l'] accumulation
                        for et in range(DT):
                            nc.tensor.matmul(
                                po[et],
                                lhsT=vp_sb[:, tt * HEADS * D + g * 512 + et * 128:
                                           tt * HEADS * D + g * 512 + (et + 1) * 128],
                                rhs=ex,
                                start=(ci == 0), stop=(ci == nchunk - 1))
                zr = zp.tile([128, 512], f32, tag="zr")
                nc.vector.reciprocal(zr, pz)
                for et in range(DT):
                    nc.vector.tensor_mul(
                        oT_sb[:, et * 2048 + ls * 512:et * 2048 + (ls + 1) * 512],
                        po[et], zr)

                # out-projection contribution of this l-slice:
                # c-tiles ct = h*DT+et for h in (h0, h0+1)
                ow_tiles = {}
                for st in range(2):
                    psc = psC.tile([128, 512], f32, tag="psC",
                                   name=f"psc{ls}_{st}")
                    for ci2, ct in enumerate(range(h0 * DT, (h0 + 2) * DT)):
                        h, et = divmod(ct, DT)
                        if st == 0:
                            ow_tiles[ct] = owp.tile([128, D], b16, tag="ow",
                                                    name=f"ow{ct}")
                            nc.sync.dma_start(
                                out=ow_tiles[ct],
                                in_=owT[ct * 128:(ct + 1) * 128, :])
                        nc.tensor.matmul(
                            psc,
                            lhsT=oT_sb[:, et * 2048 + h * S + st * 128:
                                       et * 2048 + h * S + (st + 1) * 128],
                            rhs=ow_tiles[ct],
                            start=(ci2 == 0), stop=(ci2 == 2 * DT - 1))
                    if ls == 0:
                        nc.vector.tensor_add(fin32[:, st * D:(st + 1) * D],
                                             psc, out_br_sb)
                    else:
                        nc.vector.tensor_add(fin32[:, st * D:(st + 1) * D],
                                             psc, fin32[:, st * D:(st + 1) * D])

            for st in range(2):
                nc.sync.dma_start(out=out[st * 128:(st + 1) * 128, :],
                                  in_=fin32[:, st * D:(st + 1) * D])

    nc.compile()
    return nc


def _get_program():
    if "nc" not in _CACHE:
        _CACHE["nc"] = _build_program()
    return _CACHE["nc"]


def _prep_shared(inputs):
    bf = NP_BF16
    f32c = np.ascontiguousarray
    shared = {
        "wkT": f32c(np.asarray(inputs["wk_w"], np.float32).T).astype(bf),
        "wqT": f32c(np.asarray(inputs["wq_w"], np.float32).T).astype(bf),
        "wvT": f32c(np.asarray(inputs["wv_w"], np.float32).T).astype(bf),
        "owT": f32c(np.asarray(inputs["out_w"], np.float32).T).astype(bf),
        "wk_bT": f32c(np.asarray(inputs["wk_b"], np.float32).reshape(JT, 128).T),
        "wq_bT": f32c(np.asarray(inputs["wq_b"], np.float32).reshape(JT, 128).T),
        "wv_br": f32c(np.broadcast_to(
            np.asarray(inputs["wv_b"], np.float32)[None, :], (128, HEADS * D))),
        "out_br": f32c(np.broadcast_to(
            np.asarray(inputs["out_b"], np.float32)[None, :], (128, D))),
        "ones": np.ones((128, 128), bf),
    }
    return shared


def _make_in_maps(inputs):
    bf = NP_BF16
    shared = _prep_shared(inputs)
    q = np.asarray(inputs["q"], np.float32)
    k = np.asarray(inputs["k"], np.float32)
    v = np.asarray(inputs["v"], np.float32)
    in_maps = []
    for core in range(NCORES):
        b, half = divmod(core, 2)
        m = dict(shared)
        m["qT"] = np.ascontiguousarray(q[b, half * S:(half + 1) * S, :].T).astype(bf)
        m["kT"] = np.ascontiguousarray(k[b].T).astype(bf)
        m["vT"] = np.ascontiguousarray(v[b].T).astype(bf)
        in_maps.append(m)
    return in_maps


def kernel(**inputs):
    from concourse.bass_utils import run_bass_kernel_spmd

    nc = _get_program()
    in_maps = _make_in_maps(inputs)
    res = run_bass_kernel_spmd(nc, in_maps, core_ids=list(range(NCORES)))
    _CACHE["last_results"] = res
    out = np.empty((BS, SEQ, D), np.float32)
    for core in range(NCORES):
        b, half = divmod(core, 2)
        out[b, half * S:(half + 1) * S, :] = res.results[core]["out"]
    return out


if __name__ == "__main__":
    rng = np.random.default_rng(0)
    fake = {
        "q": rng.standard_normal((BS, SEQ, D)).astype(np.float32),
        "k": rng.standard_normal((BS, SEQ, D)).astype(np.float32),
        "v": rng.standard_normal((BS, SEQ, D)).astype(np.float32),
        "wq_w": (rng.standard_normal((D * HEADS, D)) * 0.02).astype(np.float32),
        "wq_b": (rng.standard_normal((D * HEADS,)) * 0.02).astype(np.float32),
        "wk_w": (rng.standard_normal((D * HEADS, D)) * 0.02).astype(np.float32),
        "wk_b": (rng.standard_normal((D * HEADS,)) * 0.02).astype(np.float32),
        "wv_w": (rng.standard_normal((D * HEADS, D)) * 0.02).astype(np.float32),
        "wv_b": (rng.standard_normal((D * HEADS,)) * 0.02).astype(np.float32),
        "out_w": (rng.standard_normal((D, D * HEADS)) * 0.02).astype(np.float32),
        "out_b": (rng.standard_normal((D,)) * 0.02).astype(np.float32),
    }
    o = kernel(**fake)
    print("kernel ran, out shape", o.shape, "std", o.std())


# revision 26
# speedup vs baseline: 1.1467x; 1.0027x over previous
"""Trainium2 Bass kernel for nn_MultiHeadAttention_48086453846410.

Reference computation (heads folded into the sequence axis, softmax over the
FULL L = seq*heads key axis):
    qp = (q @ wk_w.T + wk_b).reshape(bs, L, d)   # note swapped wk/wq, faithful
    kp = (k @ wq_w.T + wq_b).reshape(bs, L, d)
    vp = (v @ wv_w.T + wv_b).reshape(bs, L, d)
    scores = qp @ kp.T / sqrt(d); attn = softmax(scores, -1)
    o = (attn @ vp).reshape(bs, seq, d*heads)
    out = o @ out_w.T + out_b

Sharding: 8 cores = (batch b in 0..3) x (seq half). Each core owns 256 query
seq positions of one batch (2048 query rows l' = h*256+s). Softmax is over
keys, so query rows are independent -> no collectives.

On-device layout strategy (all matmuls bf16 inputs, fp32 PSUM accumulate):
 - host pre-transposes activations/weights so no on-device transposes at all
 - qpT (interleaved d-tile-major layout so score matmuls take two heads per
   N=512 moving operand) / kpT computed transposed (proj dim j on partitions)
 - vp computed in natural layout (t on partitions)
 - scores computed transposed: scoresT[m=(g,t), l'] -> softmax needs only
   exp (scores bounded: |s| < 2, so no max subtraction) and the denominator
   Z, computed by a ones-matmul (replicated across partitions for free);
   normalization is deferred to oT columns.
 - attn@v consumes exp tiles directly as the moving operand -> oT (e on
   partitions), which is exactly the lhsT layout for the out projection.
 - out projection runs per l-slice (pipelined into phase B) with fp32
   partial sums held in SBUF via DVE adds.
"""

import math
import sys

for _p in ("/opt/trn_rl_repo",):
    if _p not in sys.path:
        sys.path.insert(0, _p)

import numpy as np
import ml_dtypes

BS, SEQ, D, HEADS = 4, 512, 512, 8
NCORES = 8
S = SEQ // 2            # 256 query seq rows per core
JT = HEADS * D // 128   # 32 tiles of the 4096 projection dim
DT = D // 128           # 4 tiles of the 512 contraction dim
TT = SEQ // 128         # 4 key-seq tiles per head
LSLICES = 4             # l' = 2048 per core, processed in 4 slices of 512
WQCOLS = 1024           # weight streaming tile width (quarter tiles)
NP_BF16 = ml_dtypes.bfloat16

_CACHE = {}


def _build_program():
    from concourse import bacc
    import concourse.mybir as mybir
    import concourse.tile as tile
    from concourse.dt import dt

    f32 = dt.float32
    b16 = dt.bfloat16
    Act = mybir.ActivationFunctionType

    nc = bacc.Bacc(None, target_bir_lowering=False, debug=False,
                   num_devices=NCORES)

    def din(name, shape, dty=b16):
        return nc.dram_tensor(name, shape, dty, kind="ExternalInput").ap()

    qT = din("qT", [D, S])                 # q[b, half].T      (d, s)
    kT = din("kT", [D, SEQ])               # k[b].T            (d, t)
    vT = din("vT", [D, SEQ])               # v[b].T            (d, t)
    wkT = din("wkT", [D, HEADS * D])       # wk_w.T            (d, j)
    wqT = din("wqT", [D, HEADS * D])       # wq_w.T            (d, j)
    wvT = din("wvT", [D, HEADS * D])       # wv_w.T            (d, j)
    owT = din("owT", [HEADS * D, D])       # out_w.T           (c, r)
    wk_bT = din("wk_bT", [128, JT], f32)   # wk_b.reshape(JT,128).T
    wq_bT = din("wq_bT", [128, JT], f32)
    wv_br = din("wv_br", [128, HEADS * D], f32)   # wv_b replicated
    out_br = din("out_br", [128, D], f32)         # out_b replicated
    ones = din("ones", [128, 128])
    out = nc.dram_tensor("out", [S, D], f32, kind="ExternalOutput").ap()

    inv_sqrt_d = 1.0 / math.sqrt(D)
    NWQ = (HEADS * D) // WQCOLS  # 4 quarter-tiles per d-tile row

    with tile.TileContext(nc) as tc:
        with (
            tc.tile_pool(name="const", bufs=1) as cp,
            tc.tile_pool(name="wpool", bufs=20) as wp,
            tc.tile_pool(name="acts", bufs=1) as acp,
            tc.tile_pool(name="state", bufs=1) as sp,
            tc.tile_pool(name="expp", bufs=6) as ep,
            tc.tile_pool(name="zrp", bufs=2) as zp,
            tc.tile_pool(name="owp", bufs=8) as owp,
            tc.tile_pool(name="psA", bufs=2, space="PSUM") as psA,
            tc.tile_pool(name="psC", bufs=1, space="PSUM") as psC,
            tc.tile_pool(name="psO", bufs=4, space="PSUM") as psO,
            tc.tile_pool(name="psZ", bufs=1, space="PSUM") as psZ,
        ):
            # ---- weight streaming: quarter tiles (128 x WQCOLS) ----
            # tile index (dt, wq) covers d rows [dt*128,...), j cols
            # [wq*WQCOLS,...). Emission order = consumption order.
            def load_w(dram, nm):
                tiles = {}
                for wq in range(NWQ):
                    for dt_ in range(DT):
                        t = wp.tile([128, WQCOLS], b16, tag="w",
                                    name=f"w_{nm}_{dt_}_{wq}")
                        eng = nc.sync if dt_ % 2 == 0 else nc.gpsimd
                        eng.dma_start(
                            out=t,
                            in_=dram[dt_ * 128:(dt_ + 1) * 128,
                                     wq * WQCOLS:(wq + 1) * WQCOLS])
                        tiles[(dt_, wq)] = t
                return tiles

            def wslice(tiles, dt_, j0, width):
                wq, off = divmod(j0, WQCOLS)
                assert off + width <= WQCOLS
                return tiles[(dt_, wq)][:, off:off + width]

            # phase-A1 critical path first: qT (small) then wk weights
            qT_sb = acp.tile([128, DT * S], b16, tag="qT")
            nc.gpsimd.dma_start(out=qT_sb.rearrange("p (t n) -> p t n", n=S),
                                in_=qT.rearrange("(t p) n -> p t n", p=128))
            wk_bT_sb = cp.tile([128, JT], f32, tag="wkb")
            nc.sync.dma_start(out=wk_bT_sb, in_=wk_bT)
            wk_sb = load_w(wkT, "k")

            kT_sb = acp.tile([128, DT * SEQ], b16, tag="kT")
            vT_sb = acp.tile([128, DT * SEQ], b16, tag="vT")
            nc.sync.dma_start(out=kT_sb.rearrange("p (t n) -> p t n", n=SEQ),
                              in_=kT.rearrange("(t p) n -> p t n", p=128))
            nc.gpsimd.dma_start(out=vT_sb.rearrange("p (t n) -> p t n", n=SEQ),
                                in_=vT.rearrange("(t p) n -> p t n", p=128))
            wq_bT_sb = cp.tile([128, JT], f32, tag="wqb")
            nc.sync.dma_start(out=wq_bT_sb, in_=wq_bT)

            # ---- persistent state ----
            # qpT interleaved: col block (dt*HEADS + h)*S
            qpT_sb = sp.tile([128, JT * S], b16, tag="qpT")       # 16KB/part
            kpT_sb = sp.tile([128, JT * SEQ], b16, tag="kpT")     # 32KB/part
            vp_sb = sp.tile([128, TT * HEADS * D], b16, tag="vp")  # 32KB/part
            oT_sb = sp.tile([128, DT * 2048], b16, tag="oT")      # 16KB/part
            fin32 = sp.tile([128, 2 * D], f32, tag="fin32")       # 4KB/part

            # ---- phase A1: qpT[j, s] = wkT.T @ qT + wk_b ----
            for jt in range(JT):
                h, dt_of_j = divmod(jt, DT)
                ps = psA.tile([128, 512], f32, tag="psA")
                for dt_ in range(DT):
                    nc.tensor.matmul(
                        ps[:, :S],
                        lhsT=wslice(wk_sb, dt_, jt * 128, 128),
                        rhs=qT_sb[:, dt_ * S:(dt_ + 1) * S],
                        start=(dt_ == 0), stop=(dt_ == DT - 1))
                nc.scalar.activation(
                    qpT_sb[:, (dt_of_j * HEADS + h) * S:
                           (dt_of_j * HEADS + h + 1) * S],
                    ps[:, :S], Act.Identity,
                    bias=wk_bT_sb[:, jt:jt + 1], scale=1.0)

            # ---- phase A2: kpT[j, t] = wqT.T @ kT + wq_b ----
            wq_sb = load_w(wqT, "q")
            for jt in range(JT):
                ps = psA.tile([128, 512], f32, tag="psA")
                for dt_ in range(DT):
                    nc.tensor.matmul(
                        ps,
                        lhsT=wslice(wq_sb, dt_, jt * 128, 128),
                        rhs=kT_sb[:, dt_ * SEQ:(dt_ + 1) * SEQ],
                        start=(dt_ == 0), stop=(dt_ == DT - 1))
                nc.scalar.activation(kpT_sb[:, jt * SEQ:(jt + 1) * SEQ], ps,
                                     Act.Identity, bias=wq_bT_sb[:, jt:jt + 1],
                                     scale=1.0)

            # ---- phase A3: vp[t, j] = vT.T @ wvT + wv_b (natural layout) ----
            wv_sb = load_w(wvT, "v")
            wv_br_sb = cp.tile([128, HEADS * D], f32, tag="wvb")
            nc.sync.dma_start(out=wv_br_sb, in_=wv_br)
            ones_sb = cp.tile([128, 128], b16, tag="ones")
            nc.sync.dma_start(out=ones_sb, in_=ones)
            out_br_sb = cp.tile([128, D], f32, tag="outb")
            nc.sync.dma_start(out=out_br_sb, in_=out_br)
            for tt in range(TT):
                for js in range(HEADS):
                    ps = psA.tile([128, 512], f32, tag="psA")
                    for dt_ in range(DT):
                        nc.tensor.matmul(
                            ps,
                            lhsT=vT_sb[:, dt_ * SEQ + tt * 128:
                                       dt_ * SEQ + (tt + 1) * 128],
                            rhs=wslice(wv_sb, dt_, js * 512, 512),
                            start=(dt_ == 0), stop=(dt_ == DT - 1))
                    nc.vector.tensor_add(
                        vp_sb[:, tt * HEADS * D + js * 512:
                              tt * HEADS * D + (js + 1) * 512],
                        ps, wv_br_sb[:, js * 512:(js + 1) * 512])

            # ---- phase B + pipelined out-projection, 4 l-slices ----
            for ls in range(LSLICES):
                h0 = 2 * ls
                pz = psZ.tile([128, 512], f32, tag="psZ", name=f"pz{ls}")
                po = [psO.tile([128, 512], f32, tag="psO", name=f"po{ls}_{i}")
                      for i in range(DT)]
                nchunk = HEADS * TT  # 32
                for g in range(HEADS):
                    for tt in range(TT):
                        ci = g * TT + tt
                        ps = psA.tile([128, 512], f32, tag="psA")
                        # scoresT[(g,tt), (h0..h0+1, s)] - both heads per MM
                        for dt_ in range(DT):
                            nc.tensor.matmul(
                                ps,
                                lhsT=kpT_sb[:, (g * DT + dt_) * SEQ + tt * 128:
                                            (g * DT + dt_) * SEQ + (tt + 1) * 128],
                                rhs=qpT_sb[:, (dt_ * HEADS + h0) * S:
                                           (dt_ * HEADS + h0 + 2) * S],
                                start=(dt_ == 0), stop=(dt_ == DT - 1))
                        ex = ep.tile([128, 512], b16, tag="exp")
                        nc.scalar.activation(ex, ps, Act.Exp, bias=0.0,
                                             scale=inv_sqrt_d)
                        # Z (col sums, replicated over partitions via ones)
                        nc.tensor.matmul(pz, lhsT=ones_sb, rhs=ex,
                                         start=(ci == 0), stop=(ci == nchunk - 1))
                        # unnormalized oT[e, l'] accumulation
                        for et in range(DT):
                            nc.tensor.matmul(
                                po[et],
                                lhsT=vp_sb[:, tt * HEADS * D + g * 512 + et * 128:
                                           tt * HEADS * D + g * 512 + (et + 1) * 128],
                                rhs=ex,
                                start=(ci == 0), stop=(ci == nchunk - 1))
                zr = zp.tile([128, 512], f32, tag="zr")
                nc.vector.reciprocal(zr, pz)
                for et in range(DT):
                    nc.vector.tensor_mul(
                        oT_sb[:, et * 2048 + ls * 512:et * 2048 + (ls + 1) * 512],
                        po[et], zr)

                # out-projection contribution of this l-slice:
                # c-tiles ct = h*DT+et for h in (h0, h0+1)
                ow_tiles = {}
                for st in range(2):
                    psc = psC.tile([128, 512], f32, tag="psC",
                                   name=f"psc{ls}_{st}")
                    for ci2, ct in enumerate(range(h0 * DT, (h0 + 2) * DT)):
                        h, et = divmod(ct, DT)
                        if st == 0:
                            ow_tiles[ct] = owp.tile([128, D], b16, tag="ow",
                                                    name=f"ow{ct}")
                            nc.sync.dma_start(
                                out=ow_tiles[ct],
                                in_=owT[ct * 128:(ct + 1) * 128, :])
                        nc.tensor.matmul(
                            psc,
                            lhsT=oT_sb[:, et * 2048 + h * S + st * 128:
                                       et * 2048 + h * S + (st + 1) * 128],
                            rhs=ow_tiles[ct],
                            start=(ci2 == 0), stop=(ci2 == 2 * DT - 1))
                    if ls == 0:
                        nc.vector.tensor_add(fin32[:, st * D:(st + 1) * D],
                                             psc, out_br_sb)
                    else:
                        nc.vector.tensor_add(fin32[:, st * D:(st + 1) * D],
                                             psc, fin32[:, st * D:(st + 1) * D])

            for st in range(2):
                nc.sync.dma_start(out=out[st * 128:(st + 1) * 128, :],
                                  in_=fin32[:, st * D:(st + 1) * D])

    nc.compile()
    return nc


def _get_program():
    if "nc" not in _CACHE:
        _CACHE["nc"] = _build_program()
    return _CACHE["nc"]


def _prep_shared(inputs):
    bf = NP_BF16
    f32c = np.ascontiguousarray
    shared = {
        "wkT": f32c(np.asarray(inputs["wk_w"], np.float32).T).astype(bf),
        "wqT": f32c(np.asarray(inputs["wq_w"], np.float32).T).astype(bf),
        "wvT": f32c(np.asarray(inputs["wv_w"], np.float32).T).astype(bf),
        "owT": f32c(np.asarray(inputs["out_w"], np.float32).T).astype(bf),
        "wk_bT": f32c(np.asarray(inputs["wk_b"], np.float32).reshape(JT, 128).T),
        "wq_bT": f32c(np.asarray(inputs["wq_b"], np.float32).reshape(JT, 128).T),
        "wv_br": f32c(np.broadcast_to(
            np.asarray(inputs["wv_b"], np.float32)[None, :], (128, HEADS * D))),
        "out_br": f32c(np.broadcast_to(
            np.asarray(inputs["out_b"], np.float32)[None, :], (128, D))),
        "ones": np.ones((128, 128), bf),
    }
    return shared


def _make_in_maps(inputs):
    bf = NP_BF16
    shared = _prep_shared(inputs)
    q = np.asarray(inputs["q"], np.float32)
    k = np.asarray(inputs["k"], np.float32)
    v = np.asarray(inputs["v"], np.float32)
    in_maps = []
    for core in range(NCORES):
        b, half = divmod(core, 2)
        m = dict(shared)
        m["qT"] = np.ascontiguousarray(q[b, half * S:(half + 1) * S, :].T).astype(bf)
        m["kT"] = np.ascontiguousarray(k[b].T).astype(bf)
        m["vT"] = np.ascontiguousarray(v[b].T).astype(bf)
        in_maps.append(m)
    return in_maps


def kernel(**inputs):
    from concourse.bass_utils import run_bass_kernel_spmd

    nc = _get_program()
    in_maps = _make_in_maps(inputs)
    res = run_bass_kernel_spmd(nc, in_maps, core_ids=list(range(NCORES)))
    _CACHE["last_results"] = res
    out = np.empty((BS, SEQ, D), np.float32)
    for core in range(NCORES):
        b, half = divmod(core, 2)
        out[b, half * S:(half + 1) * S, :] = res.results[core]["out"]
    return out


if __name__ == "__main__":
    rng = np.random.default_rng(0)
    fake = {
        "q": rng.standard_normal((BS, SEQ, D)).astype(np.float32),
        "k": rng.standard_normal((BS, SEQ, D)).astype(np.float32),
        "v": rng.standard_normal((BS, SEQ, D)).astype(np.float32),
        "wq_w": (rng.standard_normal((D * HEADS, D)) * 0.02).astype(np.float32),
        "wq_b": (rng.standard_normal((D * HEADS,)) * 0.02).astype(np.float32),
        "wk_w": (rng.standard_normal((D * HEADS, D)) * 0.02).astype(np.float32),
        "wk_b": (rng.standard_normal((D * HEADS,)) * 0.02).astype(np.float32),
        "wv_w": (rng.standard_normal((D * HEADS, D)) * 0.02).astype(np.float32),
        "wv_b": (rng.standard_normal((D * HEADS,)) * 0.02).astype(np.float32),
        "out_w": (rng.standard_normal((D, D * HEADS)) * 0.02).astype(np.float32),
        "out_b": (rng.standard_normal((D,)) * 0.02).astype(np.float32),
    }
    o = kernel(**fake)
    print("kernel ran, out shape", o.shape, "std", o.std())
